# revision 4
# baseline (speedup 1.0000x reference)
"""Trainium2 Bass kernel for nn_CausalSelfAttention_60224031424653.

Reference computation (B=4, T=4096, C=1024, H=16, D=64, sliding window 128):
  q,k,v = x@Wq.T, x@Wk.T, x@Wv.T ; v = (1-lamb)*v + lamb*v1
  q,k = rms_norm(q), rms_norm(k) (per head, over D)
  q,k = rope(q,k; cos,sin)
  out = sliding-window causal attention (each query sees previous 128 keys)
  y = out@Wproj.T ;  returns (y, v1)

Sharding (8 cores): core c = (batch b = c//2, head-group hg = c%2 of 8 heads).
Each core computes a partial y over its 512 channels' contribution to the
output projection; host sums the two partials per batch.

Per-core kernel: stream T in 128-token blocks g=0..31 with a one-block lag:
  iter g:  A) project block g -> q,k,v (fp32r matmuls, xT stationary);
              rms-norm via DVE Newton rsqrt; rope; PE-transpose q,k
           B) scores ST_{g-1} [keys of block g-1 x 256 queries] + additive
              band mask via identity matmul; exp on ACT -> PT (fp32r)
           C) attention output for query block g-1 from PT_{g-1}, PT_{g-2}
              with a fused ones-column denominator in v_aug; normalize on DVE
           D) output projection of block g-1; DMA partial y out.

fp32r (fp32 rounded to ~13 mantissa bits, exact fp32 PSUM accumulation) is
used for all matmul operands: ~4x faster than plain fp32 on the PE.
"""
import sys

sys.path.insert(0, "/opt/trn_rl_repo")

import numpy as np
import concourse.bass as bass
import concourse.mybir as mybir
import concourse.tile as tile
from concourse import bacc
from concourse.bass_utils import run_bass_kernel_spmd

F32 = mybir.dt.float32
F32R = mybir.dt.float32r
I32 = mybir.dt.int32
ALU = mybir.AluOpType
ACTF = mybir.ActivationFunctionType

T = 4096
CIN = 1024
HL = 8          # local heads per core
D = 64
COUT = HL * D   # 512
NB = T // 128   # 32 blocks
CT = CIN // 128  # 8 cin tiles
HT = COUT // 128  # 4 local cout tiles
EPS64 = 64.0 * 1e-6
MAGIC = 0x5F3759DF

_CACHE = {}


def _build():
    nc = bacc.Bacc("TRN2", target_bir_lowering=False, debug=False, num_devices=8)

    xT_d = nc.dram_tensor("xT", [CIN, T], F32R, kind="ExternalInput").ap()
    wq_d = nc.dram_tensor("wqT", [CIN, COUT], F32R, kind="ExternalInput").ap()
    wk_d = nc.dram_tensor("wkT", [CIN, COUT], F32R, kind="ExternalInput").ap()
    wv_d = nc.dram_tensor("wvT", [CIN, COUT], F32R, kind="ExternalInput").ap()
    wp_d = nc.dram_tensor("wpT", [COUT, CIN], F32R, kind="ExternalInput").ap()
    v1_d = nc.dram_tensor("v1s", [T, COUT], F32, kind="ExternalInput").ap()
    cos_d = nc.dram_tensor("cosb", [T, D], F32, kind="ExternalInput").ap()
    sin_d = nc.dram_tensor("sinb", [T, D], F32, kind="ExternalInput").ap()
    bb_d = nc.dram_tensor("bandb", [128, 256], F32R, kind="ExternalInput").ap()
    bbs_d = nc.dram_tensor("bandbsw", [128, 256], F32R, kind="ExternalInput").ap()
    idn_d = nc.dram_tensor("idn", [128, 128], F32R, kind="ExternalInput").ap()
    y_d = nc.dram_tensor("y", [T, CIN], F32, kind="ExternalOutput").ap()

    xT_v = xT_d.rearrange("(ct p) t -> p ct t", p=128)
    wq_v = wq_d.rearrange("(ct p) n -> p ct n", p=128)
    wk_v = wk_d.rearrange("(ct p) n -> p ct n", p=128)
    wv_v = wv_d.rearrange("(ct p) n -> p ct n", p=128)
    wp_v = wp_d.rearrange("(ct p) n -> p ct n", p=128)

    with tile.TileContext(nc) as tc:
        with (
            tc.tile_pool(name="wpool", bufs=1) as wpool,
            tc.tile_pool(name="ring", bufs=1) as ringp,
            tc.tile_pool(name="io", bufs=3) as io,
            tc.tile_pool(name="stg", bufs=2) as stg,
            tc.tile_pool(name="kv", bufs=3) as kvp,
            tc.tile_pool(name="vau", bufs=4) as vaup,
            tc.tile_pool(name="pt", bufs=3) as ptp,
            tc.tile_pool(name="attn", bufs=2) as att,
            tc.tile_pool(name="ps_qkv", bufs=1, space="PSUM") as ps_qkv,
            tc.tile_pool(name="ps_tr", bufs=2, space="PSUM") as ps_tr,
            tc.tile_pool(name="ps_st", bufs=1, space="PSUM") as ps_st,
            tc.tile_pool(name="ps_o", bufs=1, space="PSUM") as ps_o,
            tc.tile_pool(name="ps_y", bufs=1, space="PSUM") as ps_y,
        ):
            wq_t = wpool.tile([128, CT, COUT], F32R, name="wq_t")
            wk_t = wpool.tile([128, CT, COUT], F32R, name="wk_t")
            wv_t = wpool.tile([128, CT, COUT], F32R, name="wv_t")
            wp_t = wpool.tile([128, HT, CIN], F32R, name="wp_t")
            bb_t = wpool.tile([128, 256], F32R, name="bb_t")
            bbs_t = wpool.tile([128, 256], F32R, name="bbs_t")
            idn_t = wpool.tile([128, 128], F32R, name="idn_t")
            nc.sync.dma_start(wq_t[:], wq_v)
            nc.sync.dma_start(wk_t[:], wk_v)
            nc.sync.dma_start(wv_t[:], wv_v)
            nc.sync.dma_start(wp_t[:], wp_v)
            nc.sync.dma_start(bb_t[:], bb_d)
            nc.sync.dma_start(bbs_t[:], bbs_d)
            nc.sync.dma_start(idn_t[:], idn_d)

            # qT ring: [d-of-2-heads(128), ht, slot, t]
            qring = ringp.tile([128, HT, 2, 128], F32R, name="qring")

            kT_hist = {}    # g -> kT tile [128, HT, 128]
            va_hist = {}    # g -> v_aug tile [128, HL, 66]
            pt_hist = {}    # g -> PT tile [128, HL, 256]

            for g in range(NB + 1):
                # ---------------- stage A: project block g ----------------
                if g < NB:
                    c0 = g * 128
                    xg = io.tile([128, CT, 128], F32R, tag="xg")
                    nc.sync.dma_start(xg[:], xT_v[:, :, c0 : c0 + 128])
                    v1g = io.tile([128, COUT], F32, tag="v1g")
                    nc.sync.dma_start(v1g[:], v1_d[c0 : c0 + 128, :])
                    cosg = io.tile([128, D], F32, tag="cosg")
                    nc.sync.dma_start(cosg[:], cos_d[c0 : c0 + 128, :])
                    sing = io.tile([128, D], F32, tag="sing")
                    nc.sync.dma_start(sing[:], sin_d[c0 : c0 + 128, :])

                    q_ps = ps_qkv.tile([128, COUT], F32, tag="q_ps")
                    k_ps = ps_qkv.tile([128, COUT], F32, tag="k_ps")
                    v_ps = ps_qkv.tile([128, COUT], F32, tag="v_ps")
                    for ct in range(CT):
                        se = dict(start=(ct == 0), stop=(ct == CT - 1))
                        nc.tensor.matmul(q_ps[:], xg[:, ct, :], wq_t[:, ct, :], **se)
                        nc.tensor.matmul(k_ps[:], xg[:, ct, :], wk_t[:, ct, :], **se)
                        nc.tensor.matmul(v_ps[:], xg[:, ct, :], wv_t[:, ct, :], **se)

                    # v_aug: [t, h, 0:64]=v + v1s (pre-scaled), [t, h, 64:66]=1
                    v_aug = vaup.tile([128, HL, 66], F32R, tag="v_aug")
                    nc.vector.memset(v_aug[:, :, 64:66].bitcast(F32), 1.0)
                    nc.vector.tensor_tensor(
                        out=v_aug[:, :, 0:64],
                        in0=v_ps[:].rearrange("p (h d) -> p h d", h=HL),
                        in1=v1g[:].rearrange("p (h d) -> p h d", h=HL),
                        op=ALU.add,
                    )
                    va_hist[g] = v_aug

                    # sum of squares for q (cols 0:8) and k (cols 8:16)
                    sq = stg.tile([128, COUT], F32, tag="sq")
                    ssq = stg.tile([128, 2 * HL], F32, tag="ssq")
                    nc.scalar.activation(sq[:], q_ps[:], ACTF.Square)
                    nc.vector.reduce_sum(
                        out=ssq[:, 0:HL],
                        in_=sq[:].rearrange("p (h d) -> p h d", h=HL),
                        axis=mybir.AxisListType.X,
                    )
                    nc.scalar.activation(sq[:], k_ps[:], ACTF.Square)
                    nc.vector.reduce_sum(
                        out=ssq[:, HL : 2 * HL],
                        in_=sq[:].rearrange("p (h d) -> p h d", h=HL),
                        axis=mybir.AxisListType.X,
                    )
                    nc.vector.tensor_scalar_add(ssq[:], in0=ssq[:], scalar1=EPS64)
                    # Newton rsqrt: r = 1/sqrt(ssq); q scale=r, k scale=8r
                    rsy = stg.tile([128, 2 * HL], F32, tag="rsy")
                    rst = stg.tile([128, 2 * HL], F32, tag="rst")
                    nc.vector.tensor_scalar(
                        out=rst[:].bitcast(I32),
                        in0=ssq[:].bitcast(I32),
                        scalar1=1,
                        scalar2=None,
                        op0=ALU.logical_shift_right,
                    )
                    nc.vector.tensor_scalar(
                        out=rsy[:].bitcast(I32),
                        in0=rst[:].bitcast(I32),
                        scalar1=0xFFFFFFFF,
                        scalar2=None,
                        op0=ALU.bitwise_xor,
                    )
                    nc.vector.tensor_scalar(
                        out=rsy[:].bitcast(I32),
                        in0=rsy[:].bitcast(I32),
                        scalar1=MAGIC + 1,
                        scalar2=None,
                        op0=ALU.add,
                    )
                    for _ in range(3):
                        nc.vector.tensor_tensor(out=rst[:], in0=ssq[:], in1=rsy[:], op=ALU.mult)
                        nc.vector.tensor_tensor(out=rst[:], in0=rst[:], in1=rsy[:], op=ALU.mult)
                        nc.vector.tensor_scalar(
                            out=rst[:], in0=rst[:], scalar1=-0.5, scalar2=1.5,
                            op0=ALU.mult, op1=ALU.add,
                        )
                        nc.vector.tensor_tensor(out=rsy[:], in0=rsy[:], in1=rst[:], op=ALU.mult)

                    # normalize (q also folds the 1/8 attention scale)
                    qn = stg.tile([128, COUT], F32, tag="qn")
                    kn = stg.tile([128, COUT], F32, tag="kn")
                    nc.vector.tensor_tensor(
                        out=qn[:].rearrange("p (h d) -> p h d", h=HL),
                        in0=q_ps[:].rearrange("p (h d) -> p h d", h=HL),
                        in1=rsy[:, 0:HL].broadcast_to([128, HL, D]),
                        op=ALU.mult,
                    )
                    nc.vector.scalar_tensor_tensor(
                        out=kn[:].rearrange("p (h d) -> p h d", h=HL),
                        in0=k_ps[:].rearrange("p (h d) -> p h d", h=HL),
                        scalar=8.0,
                        in1=rsy[:, HL : 2 * HL].broadcast_to([128, HL, D]),
                        op0=ALU.mult,
                        op1=ALU.mult,
                    )

                    # rope: out = qn*cos + swap_halves(qn)*sin_signed
                    qr = stg.tile([128, COUT], F32, tag="qr")
                    kr = stg.tile([128, COUT], F32, tag="kr")
                    tmp = stg.tile([128, COUT], F32, tag="tmp")
                    for (src, dst) in ((qn, qr), (kn, kr)):
                        sv = src[:].rearrange("p (h a d) -> p h a d", a=2, d=32)
                        swapped = bass.AP(
                            tensor=sv.tensor,
                            offset=sv.offset + 32,
                            ap=[sv.ap[0], sv.ap[1], [-32, 2], [1, 32]],
                        )
                        sing_b = bass.AP(
                            tensor=sing[:].tensor,
                            offset=sing[:].offset,
                            ap=[sing[:].ap[0], [0, HL], [32, 2], [1, 32]],
                        )
                        nc.vector.tensor_tensor(
                            out=tmp[:].rearrange("p (h a d) -> p h a d", a=2, d=32),
                            in0=swapped,
                            in1=sing_b,
                            op=ALU.mult,
                        )
                        cosg_b = bass.AP(
                            tensor=cosg[:].tensor,
                            offset=cosg[:].offset,
                            ap=[cosg[:].ap[0], [0, HL], [1, D]],
                        )
                        nc.vector.tensor_tensor(
                            out=dst[:].rearrange("p (h d) -> p h d", h=HL),
                            in0=src[:].rearrange("p (h d) -> p h d", h=HL),
                            in1=cosg_b,
                            op=ALU.mult,
                        )
                        nc.vector.tensor_tensor(out=dst[:], in0=dst[:], in1=tmp[:], op=ALU.add)

                    # transpose q into ring slot g%2, k into kT tile
                    kT = kvp.tile([128, HT, 128], F32R, tag="kT")
                    for ht in range(HT):
                        tp = ps_tr.tile([128, 128], F32, tag="tp")
                        nc.tensor.transpose(tp[:], qr[:, ht * 128 : (ht + 1) * 128], idn_t[:].bitcast(F32))
                        nc.scalar.copy(qring[:, ht, g % 2, :], tp[:])
                        tp2 = ps_tr.tile([128, 128], F32, tag="tp")
                        nc.tensor.transpose(tp2[:], kr[:, ht * 128 : (ht + 1) * 128], idn_t[:].bitcast(F32))
                        nc.scalar.copy(kT[:, ht, :], tp2[:])
                    kT_hist[g] = kT

                # ---------------- stage B: scores for key block kb=g-1 ----------------
                if g >= 1:
                    kb = g - 1
                    kTb = kT_hist.pop(kb)
                    tail = kb == NB - 1
                    colsl = slice(128, 256) if tail else slice(0, 256)
                    ncols = 128 if tail else 256
                    maskt = bb_t if (kb % 2 == 0) else bbs_t
                    pt_t = ptp.tile([128, HL, 256], F32R, tag="pt_t")
                    for hp in range(HL // 2):
                        st = ps_st.tile([128, 2, 256], F32, tag="st")
                        for j in range(2):
                            h = hp * 2 + j
                            ht, r0 = h // 2, (h % 2) * 64
                            if tail:
                                qrhs = qring[r0 : r0 + 64, ht, 1, :]
                            else:
                                qrhs = qring[r0 : r0 + 64, ht, :, :]
                            nc.tensor.matmul(
                                st[:, j, colsl],
                                kTb[r0 : r0 + 64, ht, :],
                                qrhs,
                                start=True, stop=False,
                            )
                            nc.tensor.matmul(
                                st[:, j, colsl],
                                idn_t[:],
                                bb_t[:, 0:128] if tail else maskt[:],
                                start=False, stop=True,
                            )
                        nc.scalar.activation(
                            pt_t[:, hp * 2 : hp * 2 + 2, colsl],
                            st[:, :, colsl],
                            ACTF.Exp,
                        )
                    pt_hist[kb] = pt_t
                    if kb >= 2:
                        pt_hist.pop(kb - 2)
                        va_hist.pop(kb - 2)

                # ---------------- stage C+D: attention out + y for qb=g-1 ----------------
                if g >= 1:
                    qb = g - 1
                    sl = (qb % 2) * 128
                    ao = att.tile([128, HL, D], F32, tag="ao")
                    rec = att.tile([128, HL], F32, tag="rec")
                    for hq in range(2):  # head quads
                        o_ps = ps_o.tile([128, 4, 66], F32, tag="o_ps")
                        for hh in range(4):
                            h = hq * 4 + hh
                            if qb >= 1:
                                nc.tensor.matmul(
                                    o_ps[:, hh, :],
                                    pt_hist[qb - 1][:, h, sl : sl + 128],
                                    va_hist[qb - 1][:, h, :],
                                    start=True, stop=False,
                                )
                            nc.tensor.matmul(
                                o_ps[:, hh, :],
                                pt_hist[qb][:, h, sl : sl + 128],
                                va_hist[qb][:, h, :],
                                start=(qb == 0), stop=True,
                            )
                        hsl = slice(hq * 4, hq * 4 + 4)
                        nc.vector.reciprocal(
                            out=rec[:, hsl],
                            in_=o_ps[:, :, 64:65].rearrange("p h o -> p (h o)"),
                        )
                        nc.vector.tensor_tensor(
                            out=ao[:, hsl, :],
                            in0=o_ps[:, :, 0:64],
                            in1=rec[:, hsl].broadcast_to([128, 4, D]),
                            op=ALU.mult,
                        )

                    # transpose ao and project
                    aoT = att.tile([128, HT, 128], F32R, tag="aoT")
                    aov = ao[:].rearrange("p h d -> p (h d)")
                    for ht in range(HT):
                        tp3 = ps_tr.tile([128, 128], F32, tag="tp")
                        nc.tensor.transpose(tp3[:], aov[:, ht * 128 : (ht + 1) * 128], idn_t[:].bitcast(F32))
                        nc.scalar.copy(aoT[:, ht, :], tp3[:])

                    y_sb = io.tile([128, CIN], F32, tag="y_sb")
                    for half in range(2):
                        y_ps = ps_y.tile([128, 512], F32, tag="y_ps")
                        for ct in range(HT):
                            nc.tensor.matmul(
                                y_ps[:],
                                aoT[:, ct, :],
                                wp_t[:, ct, half * 512 : (half + 1) * 512],
                                start=(ct == 0), stop=(ct == HT - 1),
                            )
                        nc.scalar.copy(y_sb[:, half * 512 : (half + 1) * 512], y_ps[:])
                    nc.sync.dma_start(y_d[qb * 128 : (qb + 1) * 128, :], y_sb[:])

    nc.compile()
    return nc


def _host_prep(x, v1, cos, sin, Wq, Wk, Wv, Wproj, lamb):
    B = x.shape[0]
    # chronological band mask: rows=key pos in block kb, cols=256 queries
    tk = np.arange(128)[:, None]
    tq = np.arange(256)[None, :]
    valid = np.where(tq < 128, tq >= tk, (tq - 128) < tk)
    bandb = np.where(valid, 0.0, -1e30).astype(np.float32)
    bandbsw = np.concatenate([bandb[:, 128:], bandb[:, :128]], axis=1)
    idn = np.eye(128, dtype=np.float32)

    in_maps = []
    for core in range(8):
        b, hg = core // 2, core % 2
        rsl = slice(hg * 512, (hg + 1) * 512)
        xT = np.ascontiguousarray(x[b].T)
        wqT = np.ascontiguousarray(Wq[rsl, :].T)
        wkT = np.ascontiguousarray(Wk[rsl, :].T)
        wvT = np.ascontiguousarray((1.0 - lamb) * Wv[rsl, :].T)
        wpT = np.ascontiguousarray(Wproj[:, rsl].T)
        v1s = np.ascontiguousarray(lamb * v1[b].reshape(T, 1024)[:, rsl])
        sinb = sin[b].copy()
        sinb[:, :32] *= -1.0
        in_maps.append({
            "xT": xT, "wqT": wqT, "wkT": wkT, "wvT": wvT, "wpT": wpT,
            "v1s": v1s, "cosb": np.ascontiguousarray(cos[b]),
            "sinb": np.ascontiguousarray(sinb),
            "bandb": bandb, "bandbsw": bandbsw, "idn": idn,
        })
    return in_maps


def kernel(x, v1, cos, sin, Wq, Wk, Wv, Wproj, lamb, max_seq_length=None, **_):
    x = np.asarray(x, dtype=np.float32)
    v1 = np.asarray(v1, dtype=np.float32)
    cos = np.asarray(cos, dtype=np.float32)
    sin = np.asarray(sin, dtype=np.float32)
    Wq = np.asarray(Wq, dtype=np.float32)
    Wk = np.asarray(Wk, dtype=np.float32)
    Wv = np.asarray(Wv, dtype=np.float32)
    Wproj = np.asarray(Wproj, dtype=np.float32)
    lamb = float(np.asarray(lamb))

    if "nc" not in _CACHE:
        _CACHE["nc"] = _build()
    nc = _CACHE["nc"]

    in_maps = _host_prep(x, v1, cos, sin, Wq, Wk, Wv, Wproj, lamb)
    res = run_bass_kernel_spmd(nc, in_maps, core_ids=list(range(8)))

    B = x.shape[0]
    y = np.empty((B, T, CIN), dtype=np.float32)
    for b in range(B):
        y[b] = res.results[2 * b]["y"] + res.results[2 * b + 1]["y"]
    return y, v1


# revision 13
# speedup vs baseline: 1.0032x; 1.0032x over previous
"""Trainium2 Bass kernel for nn_CausalSelfAttention_60224031424653.

Reference computation (B=4, T=4096, C=1024, H=16, D=64, sliding window 128):
  q,k,v = x@Wq.T, x@Wk.T, x@Wv.T ; v = (1-lamb)*v + lamb*v1
  q,k = rms_norm(q), rms_norm(k) (per head, over D)
  q,k = rope(q,k; cos,sin)
  out = sliding-window causal attention (each query sees previous 128 keys)
  y = out@Wproj.T ;  returns (y, v1)

Sharding (8 cores): core c = (batch b = c//2, head-group hg = c%2 of 8 heads).
Each core computes a partial y over its 512 channels' contribution to the
output projection; host sums the two partials per batch.

Per-core kernel: stream T in 128-token blocks g=0..31 with a one-block lag:
  iter g:  A) project block g -> q,k,v (fp32r matmuls, xT stationary);
              rms-norm via DVE Newton rsqrt; rope; PE-transpose q,k
           B) scores ST_{g-1} [keys of block g-1 x 256 queries] + additive
              band mask via identity matmul; exp on ACT -> PT (fp32r)
           C) attention output for query block g-1 from PT_{g-1}, PT_{g-2}
              with a fused ones-column denominator in v_aug; normalize on DVE
           D) output projection of block g-1; DMA partial y out.

fp32r (fp32 rounded to ~13 mantissa bits, exact fp32 PSUM accumulation) is
used for all matmul operands: ~4x faster than plain fp32 on the PE.
"""
import sys

sys.path.insert(0, "/opt/trn_rl_repo")

import numpy as np
import concourse.bass as bass
import concourse.mybir as mybir
import concourse.tile as tile
from concourse import bacc
from concourse.bass_utils import run_bass_kernel_spmd

F32 = mybir.dt.float32
F32R = mybir.dt.float32r
I32 = mybir.dt.int32
ALU = mybir.AluOpType
ACTF = mybir.ActivationFunctionType

T = 4096
CIN = 1024
HL = 8          # local heads per core
D = 64
COUT = HL * D   # 512
NB = T // 128   # 32 blocks
CT = CIN // 128  # 8 cin tiles
HT = COUT // 128  # 4 local cout tiles
EPS64 = 64.0 * 1e-6
MAGIC = 0x5F3759DF

_CACHE = {}


def _build(maskmode="mm"):
    nc = bacc.Bacc("TRN2", target_bir_lowering=False, debug=False, num_devices=8)

    xT_d = nc.dram_tensor("xT", [CIN, T], F32R, kind="ExternalInput").ap()
    wq_d = nc.dram_tensor("wqT", [CIN, COUT], F32R, kind="ExternalInput").ap()
    wk_d = nc.dram_tensor("wkT", [CIN, COUT], F32R, kind="ExternalInput").ap()
    wv_d = nc.dram_tensor("wvT", [CIN, COUT], F32R, kind="ExternalInput").ap()
    wp_d = nc.dram_tensor("wpT", [COUT, CIN], F32R, kind="ExternalInput").ap()
    v1_d = nc.dram_tensor("v1s", [T, COUT], F32, kind="ExternalInput").ap()
    cos_d = nc.dram_tensor("cosb", [T, D], F32, kind="ExternalInput").ap()
    sin_d = nc.dram_tensor("sinb", [T, D], F32, kind="ExternalInput").ap()
    bb_d = nc.dram_tensor("bandb", [128, 256], F32R, kind="ExternalInput").ap()
    bbs_d = nc.dram_tensor("bandbsw", [128, 256], F32R, kind="ExternalInput").ap()
    idn_d = nc.dram_tensor("idn", [128, 128], F32R, kind="ExternalInput").ap()
    y_d = nc.dram_tensor("y", [T, CIN], F32, kind="ExternalOutput").ap()

    xT_v = xT_d.rearrange("(ct p) t -> p ct t", p=128)
    wq_v = wq_d.rearrange("(ct p) n -> p ct n", p=128)
    wk_v = wk_d.rearrange("(ct p) n -> p ct n", p=128)
    wv_v = wv_d.rearrange("(ct p) n -> p ct n", p=128)
    wp_v = wp_d.rearrange("(ct p) n -> p ct n", p=128)

    with tile.TileContext(nc) as tc:
        with (
            tc.tile_pool(name="wpool", bufs=1) as wpool,
            tc.tile_pool(name="ring", bufs=1) as ringp,
            tc.tile_pool(name="io", bufs=3) as io,
            tc.tile_pool(name="stg", bufs=2) as stg,
            tc.tile_pool(name="kv", bufs=3) as kvp,
            tc.tile_pool(name="vau", bufs=4) as vaup,
            tc.tile_pool(name="pt", bufs=3) as ptp,
            tc.tile_pool(name="attn", bufs=2) as att,
            tc.tile_pool(name="ps_qkv", bufs=1, space="PSUM") as ps_qkv,
            tc.tile_pool(name="ps_tr", bufs=2, space="PSUM") as ps_tr,
            tc.tile_pool(name="ps_st", bufs=1, space="PSUM") as ps_st,
            tc.tile_pool(name="ps_o", bufs=1, space="PSUM") as ps_o,
            tc.tile_pool(name="ps_y", bufs=1, space="PSUM") as ps_y,
        ):
            wq_t = wpool.tile([128, CT, COUT], F32R, name="wq_t")
            wk_t = wpool.tile([128, CT, COUT], F32R, name="wk_t")
            wv_t = wpool.tile([128, CT, COUT], F32R, name="wv_t")
            wp_t = wpool.tile([128, HT, CIN], F32R, name="wp_t")
            bb_t = wpool.tile([128, 256], F32R, name="bb_t")
            bbs_t = wpool.tile([128, 256], F32R, name="bbs_t")
            idn_t = wpool.tile([128, 128], F32R, name="idn_t")
            nc.sync.dma_start(wq_t[:], wq_v)
            nc.sync.dma_start(wk_t[:], wk_v)
            nc.sync.dma_start(wv_t[:], wv_v)
            nc.sync.dma_start(wp_t[:], wp_v)
            nc.sync.dma_start(bb_t[:], bb_d)
            nc.sync.dma_start(bbs_t[:], bbs_d)
            nc.sync.dma_start(idn_t[:], idn_d)

            # qT ring: [d-of-2-heads(128), ht, slot, t]
            qring = ringp.tile([128, HT, 2, 128], F32R, name="qring")

            kT_hist = {}    # g -> kT tile [128, HT, 128]
            va_hist = {}    # g -> v_aug tile [128, HL, 66]
            pt_hist = {}    # g -> PT tile [128, HL, 256]

            for g in range(NB + 1):
                # ---------------- stage A: project block g ----------------
                if g < NB:
                    c0 = g * 128
                    xg = io.tile([128, CT, 128], F32R, tag="xg")
                    nc.sync.dma_start(xg[:], xT_v[:, :, c0 : c0 + 128])
                    v1g = io.tile([128, COUT], F32, tag="v1g")
                    nc.sync.dma_start(v1g[:], v1_d[c0 : c0 + 128, :])
                    cosg = io.tile([128, D], F32, tag="cosg")
                    nc.sync.dma_start(cosg[:], cos_d[c0 : c0 + 128, :])
                    sing = io.tile([128, D], F32, tag="sing")
                    nc.sync.dma_start(sing[:], sin_d[c0 : c0 + 128, :])

                    q_ps = ps_qkv.tile([128, COUT], F32, tag="q_ps")
                    k_ps = ps_qkv.tile([128, COUT], F32, tag="k_ps")
                    v_ps = ps_qkv.tile([128, COUT], F32, tag="v_ps")
                    for ct in range(CT):
                        se = dict(start=(ct == 0), stop=(ct == CT - 1))
                        nc.tensor.matmul(q_ps[:], xg[:, ct, :], wq_t[:, ct, :], **se)
                        nc.tensor.matmul(k_ps[:], xg[:, ct, :], wk_t[:, ct, :], **se)
                        nc.tensor.matmul(v_ps[:], xg[:, ct, :], wv_t[:, ct, :], **se)

                    # v_aug: [t, h, 0:64]=v + v1s (pre-scaled), [t, h, 64:66]=1
                    v_aug = vaup.tile([128, HL, 66], F32R, tag="v_aug")
                    nc.vector.memset(v_aug[:, :, 64:66].bitcast(F32), 1.0)
                    nc.vector.tensor_tensor(
                        out=v_aug[:, :, 0:64],
                        in0=v_ps[:].rearrange("p (h d) -> p h d", h=HL),
                        in1=v1g[:].rearrange("p (h d) -> p h d", h=HL),
                        op=ALU.add,
                    )
                    va_hist[g] = v_aug

                    # sum of squares for q (cols 0:8) and k (cols 8:16)
                    sq = stg.tile([128, COUT], F32, tag="sq")
                    ssq = stg.tile([128, 2 * HL], F32, tag="ssq")
                    nc.scalar.activation(sq[:], q_ps[:], ACTF.Square)
                    nc.vector.reduce_sum(
                        out=ssq[:, 0:HL],
                        in_=sq[:].rearrange("p (h d) -> p h d", h=HL),
                        axis=mybir.AxisListType.X,
                    )
                    nc.scalar.activation(sq[:], k_ps[:], ACTF.Square)
                    nc.vector.reduce_sum(
                        out=ssq[:, HL : 2 * HL],
                        in_=sq[:].rearrange("p (h d) -> p h d", h=HL),
                        axis=mybir.AxisListType.X,
                    )
                    nc.vector.tensor_scalar_add(ssq[:], in0=ssq[:], scalar1=EPS64)
                    # Newton rsqrt: r = 1/sqrt(ssq); q scale=r, k scale=8r
                    rsy = stg.tile([128, 2 * HL], F32, tag="rsy")
                    rst = stg.tile([128, 2 * HL], F32, tag="rst")
                    nc.vector.tensor_scalar(
                        out=rst[:].bitcast(I32),
                        in0=ssq[:].bitcast(I32),
                        scalar1=1,
                        scalar2=None,
                        op0=ALU.logical_shift_right,
                    )
                    nc.vector.tensor_scalar(
                        out=rsy[:].bitcast(I32),
                        in0=rst[:].bitcast(I32),
                        scalar1=-1,
                        scalar2=None,
                        op0=ALU.bitwise_xor,
                    )
                    nc.vector.tensor_scalar(
                        out=rsy[:].bitcast(I32),
                        in0=rsy[:].bitcast(I32),
                        scalar1=MAGIC + 1,
                        scalar2=None,
                        op0=ALU.add,
                    )
                    for _ in range(3):
                        nc.vector.tensor_tensor(out=rst[:], in0=ssq[:], in1=rsy[:], op=ALU.mult)
                        nc.vector.tensor_tensor(out=rst[:], in0=rst[:], in1=rsy[:], op=ALU.mult)
                        nc.vector.tensor_scalar(
                            out=rst[:], in0=rst[:], scalar1=-0.5, scalar2=1.5,
                            op0=ALU.mult, op1=ALU.add,
                        )
                        nc.vector.tensor_tensor(out=rsy[:], in0=rsy[:], in1=rst[:], op=ALU.mult)

                    # normalize (q also folds the 1/8 attention scale)
                    qn = stg.tile([128, COUT], F32, tag="qn")
                    kn = stg.tile([128, COUT], F32, tag="kn")
                    nc.vector.tensor_tensor(
                        out=qn[:].rearrange("p (h d) -> p h d", h=HL),
                        in0=q_ps[:].rearrange("p (h d) -> p h d", h=HL),
                        in1=rsy[:, 0:HL].broadcast_to([128, HL, D]),
                        op=ALU.mult,
                    )
                    nc.vector.scalar_tensor_tensor(
                        out=kn[:].rearrange("p (h d) -> p h d", h=HL),
                        in0=k_ps[:].rearrange("p (h d) -> p h d", h=HL),
                        scalar=8.0,
                        in1=rsy[:, HL : 2 * HL].broadcast_to([128, HL, D]),
                        op0=ALU.mult,
                        op1=ALU.mult,
                    )

                    # rope: out = qn*cos + swap_halves(qn)*sin_signed
                    qr = stg.tile([128, COUT], F32, tag="qr")
                    kr = stg.tile([128, COUT], F32, tag="kr")
                    tmp = stg.tile([128, COUT], F32, tag="tmp")
                    for (src, dst) in ((qn, qr), (kn, kr)):
                        sv = src[:].rearrange("p (h a d) -> p h a d", a=2, d=32)
                        swapped = bass.AP(
                            tensor=sv.tensor,
                            offset=sv.offset + 32,
                            ap=[sv.ap[0], sv.ap[1], [-32, 2], [1, 32]],
                        )
                        sing_b = bass.AP(
                            tensor=sing[:].tensor,
                            offset=sing[:].offset,
                            ap=[sing[:].ap[0], [0, HL], [32, 2], [1, 32]],
                        )
                        nc.vector.tensor_tensor(
                            out=tmp[:].rearrange("p (h a d) -> p h a d", a=2, d=32),
                            in0=swapped,
                            in1=sing_b,
                            op=ALU.mult,
                        )
                        cosg_b = bass.AP(
                            tensor=cosg[:].tensor,
                            offset=cosg[:].offset,
                            ap=[cosg[:].ap[0], [0, HL], [1, D]],
                        )
                        nc.vector.tensor_tensor(
                            out=dst[:].rearrange("p (h d) -> p h d", h=HL),
                            in0=src[:].rearrange("p (h d) -> p h d", h=HL),
                            in1=cosg_b,
                            op=ALU.mult,
                        )
                        nc.vector.tensor_tensor(out=dst[:], in0=dst[:], in1=tmp[:], op=ALU.add)

                    # transpose q into ring slot g%2, k into kT tile
                    kT = kvp.tile([128, HT, 128], F32R, tag="kT")
                    for ht in range(HT):
                        tp = ps_tr.tile([128, 128], F32, tag="tp")
                        nc.tensor.transpose(tp[:], qr[:, ht * 128 : (ht + 1) * 128], idn_t[:].bitcast(F32))
                        nc.scalar.copy(qring[:, ht, g % 2, :], tp[:])
                        tp2 = ps_tr.tile([128, 128], F32, tag="tp")
                        nc.tensor.transpose(tp2[:], kr[:, ht * 128 : (ht + 1) * 128], idn_t[:].bitcast(F32))
                        nc.scalar.copy(kT[:, ht, :], tp2[:])
                    kT_hist[g] = kT

                # ---------------- stage B: scores for key block kb=g-1 ----------------
                if g >= 1:
                    kb = g - 1
                    kTb = kT_hist.pop(kb)
                    tail = kb == NB - 1
                    colsl = slice(128, 256) if tail else slice(0, 256)
                    ncols = 128 if tail else 256
                    maskt = bb_t if (kb % 2 == 0) else bbs_t
                    pt_t = ptp.tile([128, HL, 256], F32R, tag="pt_t")
                    for hp in range(HL // 2):
                        st = ps_st.tile([128, 2, 256], F32, tag="st")
                        for j in range(2):
                            h = hp * 2 + j
                            ht, r0 = h // 2, (h % 2) * 64
                            if tail:
                                qrhs = qring[r0 : r0 + 64, ht, 1, :]
                            else:
                                qrhs = qring[r0 : r0 + 64, ht, :, :]
                            nc.tensor.matmul(
                                st[:, j, colsl],
                                kTb[r0 : r0 + 64, ht, :],
                                qrhs,
                                start=True, stop=(maskmode != "mm"),
                            )
                            if maskmode == "mm":
                                nc.tensor.matmul(
                                    st[:, j, colsl],
                                    idn_t[:],
                                    bb_t[:, 0:128] if tail else maskt[:],
                                    start=False, stop=True,
                                )
                        if maskmode == "dve2":
                            scr = ptp.tile([128, 2, 256], F32, tag="scr")
                            nc.scalar.activation(scr[:, :, colsl], st[:, :, colsl], ACTF.Exp)
                            msk = bb_t[:, 0:128] if tail else maskt[:]
                            for j in range(2):
                                nc.vector.tensor_mul(
                                    pt_t[:, hp * 2 + j, colsl],
                                    scr[:, j, colsl],
                                    msk.bitcast(F32),
                                )
                        else:
                            nc.scalar.activation(
                                pt_t[:, hp * 2 : hp * 2 + 2, colsl],
                                st[:, :, colsl],
                                ACTF.Exp,
                            )
                        if maskmode == "dve":
                            msk = bb_t[:, 0:128] if tail else maskt[:]
                            for j in range(2):
                                nc.vector.tensor_mul(
                                    pt_t[:, hp * 2 + j, colsl],
                                    pt_t[:, hp * 2 + j, colsl],
                                    msk,
                                )
                        elif maskmode == "gps":
                            msk = bb_t[:, 0:128] if tail else maskt[:]
                            for j in range(2):
                                nc.gpsimd.tensor_mul(
                                    pt_t[:, hp * 2 + j, colsl],
                                    pt_t[:, hp * 2 + j, colsl],
                                    msk,
                                )
                    pt_hist[kb] = pt_t
                    if kb >= 2:
                        pt_hist.pop(kb - 2)
                        va_hist.pop(kb - 2)

                # ---------------- stage C+D: attention out + y for qb=g-1 ----------------
                if g >= 1:
                    qb = g - 1
                    sl = (qb % 2) * 128
                    ao = att.tile([128, HL, D], F32, tag="ao")
                    rec = att.tile([128, HL], F32, tag="rec")
                    for hq in range(2):  # head quads
                        o_ps = ps_o.tile([128, 4, 66], F32, tag="o_ps", name="o_ps")
                        for hh in range(4):
                            h = hq * 4 + hh
                            if qb >= 1:
                                nc.tensor.matmul(
                                    o_ps[:, hh, :],
                                    pt_hist[qb - 1][:, h, sl : sl + 128],
                                    va_hist[qb - 1][:, h, :],
                                    start=True, stop=False,
                                )
                            nc.tensor.matmul(
                                o_ps[:, hh, :],
                                pt_hist[qb][:, h, sl : sl + 128],
                                va_hist[qb][:, h, :],
                                start=(qb == 0), stop=True,
                            )
                        hsl = slice(hq * 4, hq * 4 + 4)
                        nc.vector.reciprocal(
                            out=rec[:, hsl],
                            in_=o_ps[:, :, 64:65].rearrange("p h o -> p (h o)"),
                        )
                        nc.vector.tensor_tensor(
                            out=ao[:, hsl, :],
                            in0=o_ps[:, :, 0:64],
                            in1=rec[:, hsl].broadcast_to([128, 4, D]),
                            op=ALU.mult,
                        )

                    # transpose ao and project
                    aoT = att.tile([128, HT, 128], F32R, tag="aoT")
                    aov = ao[:].rearrange("p h d -> p (h d)")
                    for ht in range(HT):
                        tp3 = ps_tr.tile([128, 128], F32, tag="tp")
                        nc.tensor.transpose(tp3[:], aov[:, ht * 128 : (ht + 1) * 128], idn_t[:].bitcast(F32))
                        nc.scalar.copy(aoT[:, ht, :], tp3[:])

                    y_sb = io.tile([128, CIN], F32, tag="y_sb")
                    for half in range(2):
                        y_ps = ps_y.tile([128, 512], F32, tag="y_ps", name="y_ps")
                        for ct in range(HT):
                            nc.tensor.matmul(
                                y_ps[:],
                                aoT[:, ct, :],
                                wp_t[:, ct, half * 512 : (half + 1) * 512],
                                start=(ct == 0), stop=(ct == HT - 1),
                            )
                        nc.scalar.copy(y_sb[:, half * 512 : (half + 1) * 512], y_ps[:])
                    nc.sync.dma_start(y_d[qb * 128 : (qb + 1) * 128, :], y_sb[:])

    nc.compile()
    return nc


def _host_prep(x, v1, cos, sin, Wq, Wk, Wv, Wproj, lamb, maskmode="mm"):
    B = x.shape[0]
    # chronological band mask: rows=key pos in block kb, cols=256 queries
    tk = np.arange(128)[:, None]
    tq = np.arange(256)[None, :]
    valid = np.where(tq < 128, tq >= tk, (tq - 128) < tk)
    if maskmode == "mm":
        bandb = np.where(valid, 0.0, -1e30).astype(np.float32)
    else:
        bandb = valid.astype(np.float32)
    bandbsw = np.concatenate([bandb[:, 128:], bandb[:, :128]], axis=1)
    idn = np.eye(128, dtype=np.float32)

    in_maps = []
    for core in range(8):
        b, hg = core // 2, core % 2
        rsl = slice(hg * 512, (hg + 1) * 512)
        xT = np.ascontiguousarray(x[b].T)
        wqT = np.ascontiguousarray(Wq[rsl, :].T)
        wkT = np.ascontiguousarray(Wk[rsl, :].T)
        wvT = np.ascontiguousarray((1.0 - lamb) * Wv[rsl, :].T)
        wpT = np.ascontiguousarray(Wproj[:, rsl].T)
        v1s = np.ascontiguousarray(lamb * v1[b].reshape(T, 1024)[:, rsl])
        sinb = sin[b].copy()
        sinb[:, :32] *= -1.0
        in_maps.append({
            "xT": xT, "wqT": wqT, "wkT": wkT, "wvT": wvT, "wpT": wpT,
            "v1s": v1s, "cosb": np.ascontiguousarray(cos[b]),
            "sinb": np.ascontiguousarray(sinb),
            "bandb": bandb, "bandbsw": bandbsw, "idn": idn,
        })
    return in_maps


def kernel(x, v1, cos, sin, Wq, Wk, Wv, Wproj, lamb, max_seq_length=None, **_):
    x = np.asarray(x, dtype=np.float32)
    v1 = np.asarray(v1, dtype=np.float32)
    cos = np.asarray(cos, dtype=np.float32)
    sin = np.asarray(sin, dtype=np.float32)
    Wq = np.asarray(Wq, dtype=np.float32)
    Wk = np.asarray(Wk, dtype=np.float32)
    Wv = np.asarray(Wv, dtype=np.float32)
    Wproj = np.asarray(Wproj, dtype=np.float32)
    lamb = float(np.asarray(lamb))

    if "nc" not in _CACHE:
        _CACHE["nc"] = _build()
    nc = _CACHE["nc"]

    in_maps = _host_prep(x, v1, cos, sin, Wq, Wk, Wv, Wproj, lamb)
    res = run_bass_kernel_spmd(nc, in_maps, core_ids=list(range(8)))

    B = x.shape[0]
    y = np.empty((B, T, CIN), dtype=np.float32)
    for b in range(B):
        y[b] = res.results[2 * b]["y"] + res.results[2 * b + 1]["y"]
    return y, v1


# revision 14
# speedup vs baseline: 1.1354x; 1.1318x over previous
"""Trainium2 Bass kernel for nn_CausalSelfAttention_60224031424653.

Reference computation (B=4, T=4096, C=1024, H=16, D=64, sliding window 128):
  q,k,v = x@Wq.T, x@Wk.T, x@Wv.T ; v = (1-lamb)*v + lamb*v1
  q,k = rms_norm(q), rms_norm(k) (per head, over D)
  q,k = rope(q,k; cos,sin)
  out = sliding-window causal attention (each query sees previous 128 keys)
  y = out@Wproj.T ;  returns (y, v1)

Sharding (8 cores): core c = (batch b = c//2, head-group hg = c%2 of 8 heads).
Each core computes a partial y over its 512 channels' contribution to the
output projection; host sums the two partials per batch.

Per-core kernel: stream T in 128-token blocks g=0..31 with a one-block lag:
  iter g:  A) project block g -> q,k,v (fp32r matmuls, xT stationary);
              rms-norm via DVE Newton rsqrt; rope; PE-transpose q,k
           B) scores ST_{g-1} [keys of block g-1 x 256 queries] + additive
              band mask via identity matmul; exp on ACT -> PT (fp32r)
           C) attention output for query block g-1 from PT_{g-1}, PT_{g-2}
              with a fused ones-column denominator in v_aug; normalize on DVE
           D) output projection of block g-1; DMA partial y out.

fp32r (fp32 rounded to ~13 mantissa bits, exact fp32 PSUM accumulation) is
used for all matmul operands: ~4x faster than plain fp32 on the PE.
"""
import sys

sys.path.insert(0, "/opt/trn_rl_repo")

import numpy as np
import concourse.bass as bass
import concourse.mybir as mybir
import concourse.tile as tile
from concourse import bacc
from concourse.bass_utils import run_bass_kernel_spmd

F32 = mybir.dt.float32
F32R = mybir.dt.float32r
I32 = mybir.dt.int32
ALU = mybir.AluOpType
ACTF = mybir.ActivationFunctionType

T = 4096
CIN = 1024
HL = 8          # local heads per core
D = 64
COUT = HL * D   # 512
NB = T // 128   # 32 blocks
CT = CIN // 128  # 8 cin tiles
HT = COUT // 128  # 4 local cout tiles
EPS64 = 64.0 * 1e-6
MAGIC = 0x5F3759DF

_CACHE = {}


def _build(maskmode="mm"):
    nc = bacc.Bacc("TRN2", target_bir_lowering=False, debug=False, num_devices=8)

    xT_d = nc.dram_tensor("xT", [CIN, T], F32R, kind="ExternalInput").ap()
    wq_d = nc.dram_tensor("wqT", [CIN, COUT], F32R, kind="ExternalInput").ap()
    wk_d = nc.dram_tensor("wkT", [CIN, COUT], F32R, kind="ExternalInput").ap()
    wv_d = nc.dram_tensor("wvT", [CIN, COUT], F32R, kind="ExternalInput").ap()
    wp_d = nc.dram_tensor("wpT", [COUT, CIN], F32R, kind="ExternalInput").ap()
    v1_d = nc.dram_tensor("v1s", [T, COUT], F32, kind="ExternalInput").ap()
    cos_d = nc.dram_tensor("cosb", [T, D], F32, kind="ExternalInput").ap()
    sin_d = nc.dram_tensor("sinb", [T, D], F32, kind="ExternalInput").ap()
    bb_d = nc.dram_tensor("bandb", [128, 256], F32R, kind="ExternalInput").ap()
    bbs_d = nc.dram_tensor("bandbsw", [128, 256], F32R, kind="ExternalInput").ap()
    idn_d = nc.dram_tensor("idn", [128, 128], F32R, kind="ExternalInput").ap()
    y_d = nc.dram_tensor("y", [T, CIN], F32, kind="ExternalOutput").ap()

    xT_v = xT_d.rearrange("(ct p) t -> p ct t", p=128)
    wq_v = wq_d.rearrange("(ct p) n -> p ct n", p=128)
    wk_v = wk_d.rearrange("(ct p) n -> p ct n", p=128)
    wv_v = wv_d.rearrange("(ct p) n -> p ct n", p=128)
    wp_v = wp_d.rearrange("(ct p) n -> p ct n", p=128)

    with tile.TileContext(nc) as tc:
        with (
            tc.tile_pool(name="wpool", bufs=1) as wpool,
            tc.tile_pool(name="ring", bufs=1) as ringp,
            tc.tile_pool(name="io", bufs=3) as io,
            tc.tile_pool(name="stg", bufs=2) as stg,
            tc.tile_pool(name="kv", bufs=3) as kvp,
            tc.tile_pool(name="vau", bufs=4) as vaup,
            tc.tile_pool(name="pt", bufs=3) as ptp,
            tc.tile_pool(name="attn", bufs=2) as att,
            tc.tile_pool(name="ps_qkv", bufs=1, space="PSUM") as ps_qkv,
            tc.tile_pool(name="ps_tr", bufs=2, space="PSUM") as ps_tr,
            tc.tile_pool(name="ps_st", bufs=2, space="PSUM") as ps_st,
            tc.tile_pool(name="ps_oy", bufs=1, space="PSUM") as ps_oy,
        ):
            wq_t = wpool.tile([128, CT, COUT], F32R, name="wq_t")
            wk_t = wpool.tile([128, CT, COUT], F32R, name="wk_t")
            wv_t = wpool.tile([128, CT, COUT], F32R, name="wv_t")
            wp_t = wpool.tile([128, HT, CIN], F32R, name="wp_t")
            bb_t = wpool.tile([128, 256], F32R, name="bb_t")
            bbs_t = wpool.tile([128, 256], F32R, name="bbs_t")
            idn_t = wpool.tile([128, 128], F32R, name="idn_t")
            nc.sync.dma_start(wq_t[:], wq_v)
            nc.sync.dma_start(wk_t[:], wk_v)
            nc.sync.dma_start(wv_t[:], wv_v)
            nc.sync.dma_start(wp_t[:], wp_v)
            nc.sync.dma_start(bb_t[:], bb_d)
            nc.sync.dma_start(bbs_t[:], bbs_d)
            nc.sync.dma_start(idn_t[:], idn_d)

            # qT ring: [d-of-2-heads(128), ht, slot, t]
            qring = ringp.tile([128, HT, 2, 128], F32R, name="qring")

            kT_hist = {}    # g -> kT tile [128, HT, 128]
            va_hist = {}    # g -> v_aug tile [128, HL, 66]
            pt_hist = {}    # g -> PT tile [128, HL, 256]

            for g in range(NB + 1):
                # ---------------- stage A: project block g ----------------
                if g < NB:
                    c0 = g * 128
                    xg = io.tile([128, CT, 128], F32R, tag="xg")
                    nc.sync.dma_start(xg[:], xT_v[:, :, c0 : c0 + 128])
                    v1g = io.tile([128, COUT], F32, tag="v1g")
                    nc.sync.dma_start(v1g[:], v1_d[c0 : c0 + 128, :])
                    cosg = io.tile([128, D], F32, tag="cosg")
                    nc.sync.dma_start(cosg[:], cos_d[c0 : c0 + 128, :])
                    sing = io.tile([128, D], F32, tag="sing")
                    nc.sync.dma_start(sing[:], sin_d[c0 : c0 + 128, :])

                    q_ps = ps_qkv.tile([128, COUT], F32, tag="q_ps")
                    k_ps = ps_qkv.tile([128, COUT], F32, tag="k_ps")
                    v_ps = ps_qkv.tile([128, COUT], F32, tag="v_ps")
                    for ct in range(CT):
                        se = dict(start=(ct == 0), stop=(ct == CT - 1))
                        nc.tensor.matmul(q_ps[:], xg[:, ct, :], wq_t[:, ct, :], **se)
                        nc.tensor.matmul(k_ps[:], xg[:, ct, :], wk_t[:, ct, :], **se)
                        nc.tensor.matmul(v_ps[:], xg[:, ct, :], wv_t[:, ct, :], **se)

                    # v_aug: [t, h, 0:64]=v + v1s (pre-scaled), [t, h, 64:66]=1
                    v_aug = vaup.tile([128, HL, 66], F32R, tag="v_aug")
                    nc.vector.memset(v_aug[:, :, 64:66].bitcast(F32), 1.0)
                    nc.vector.tensor_tensor(
                        out=v_aug[:, :, 0:64],
                        in0=v_ps[:].rearrange("p (h d) -> p h d", h=HL),
                        in1=v1g[:].rearrange("p (h d) -> p h d", h=HL),
                        op=ALU.add,
                    )
                    va_hist[g] = v_aug

                    # copy q,k to SBUF so the PSUM banks free quickly
                    qsb = stg.tile([128, COUT], F32, tag="qsb")
                    ksb = stg.tile([128, COUT], F32, tag="ksb")
                    nc.scalar.copy(qsb[:], q_ps[:])
                    nc.scalar.copy(ksb[:], k_ps[:])
                    # sum of squares for q (cols 0:8) and k (cols 8:16)
                    sq = stg.tile([128, COUT], F32, tag="sq")
                    ssq = stg.tile([128, 2 * HL], F32, tag="ssq")
                    nc.vector.tensor_tensor(out=sq[:], in0=qsb[:], in1=qsb[:], op=ALU.mult)
                    nc.vector.reduce_sum(
                        out=ssq[:, 0:HL],
                        in_=sq[:].rearrange("p (h d) -> p h d", h=HL),
                        axis=mybir.AxisListType.X,
                    )
                    nc.vector.tensor_tensor(out=sq[:], in0=ksb[:], in1=ksb[:], op=ALU.mult)
                    nc.vector.reduce_sum(
                        out=ssq[:, HL : 2 * HL],
                        in_=sq[:].rearrange("p (h d) -> p h d", h=HL),
                        axis=mybir.AxisListType.X,
                    )
                    nc.vector.tensor_scalar_add(ssq[:], in0=ssq[:], scalar1=EPS64)
                    # Newton rsqrt: r = 1/sqrt(ssq); q scale=r, k scale=8r
                    rsy = stg.tile([128, 2 * HL], F32, tag="rsy")
                    rst = stg.tile([128, 2 * HL], F32, tag="rst")
                    nc.vector.tensor_scalar(
                        out=rst[:].bitcast(I32),
                        in0=ssq[:].bitcast(I32),
                        scalar1=1,
                        scalar2=None,
                        op0=ALU.logical_shift_right,
                    )
                    nc.vector.tensor_scalar(
                        out=rsy[:].bitcast(I32),
                        in0=rst[:].bitcast(I32),
                        scalar1=-1,
                        scalar2=None,
                        op0=ALU.bitwise_xor,
                    )
                    nc.vector.tensor_scalar(
                        out=rsy[:].bitcast(I32),
                        in0=rsy[:].bitcast(I32),
                        scalar1=MAGIC + 1,
                        scalar2=None,
                        op0=ALU.add,
                    )
                    for _ in range(3):
                        nc.vector.tensor_tensor(out=rst[:], in0=ssq[:], in1=rsy[:], op=ALU.mult)
                        nc.vector.tensor_tensor(out=rst[:], in0=rst[:], in1=rsy[:], op=ALU.mult)
                        nc.vector.tensor_scalar(
                            out=rst[:], in0=rst[:], scalar1=-0.5, scalar2=1.5,
                            op0=ALU.mult, op1=ALU.add,
                        )
                        nc.vector.tensor_tensor(out=rsy[:], in0=rsy[:], in1=rst[:], op=ALU.mult)

                    # normalize (q also folds the 1/8 attention scale)
                    qn = stg.tile([128, COUT], F32, tag="qn")
                    kn = stg.tile([128, COUT], F32, tag="kn")
                    nc.vector.tensor_tensor(
                        out=qn[:].rearrange("p (h d) -> p h d", h=HL),
                        in0=qsb[:].rearrange("p (h d) -> p h d", h=HL),
                        in1=rsy[:, 0:HL].broadcast_to([128, HL, D]),
                        op=ALU.mult,
                    )
                    nc.vector.scalar_tensor_tensor(
                        out=kn[:].rearrange("p (h d) -> p h d", h=HL),
                        in0=ksb[:].rearrange("p (h d) -> p h d", h=HL),
                        scalar=8.0,
                        in1=rsy[:, HL : 2 * HL].broadcast_to([128, HL, D]),
                        op0=ALU.mult,
                        op1=ALU.mult,
                    )

                    # rope: out = qn*cos + swap_halves(qn)*sin_signed
                    qr = stg.tile([128, COUT], F32, tag="qr")
                    kr = stg.tile([128, COUT], F32, tag="kr")
                    tmp = stg.tile([128, COUT], F32, tag="tmp")
                    for (src, dst) in ((qn, qr), (kn, kr)):
                        sv = src[:].rearrange("p (h a d) -> p h a d", a=2, d=32)
                        swapped = bass.AP(
                            tensor=sv.tensor,
                            offset=sv.offset + 32,
                            ap=[sv.ap[0], sv.ap[1], [-32, 2], [1, 32]],
                        )
                        sing_b = bass.AP(
                            tensor=sing[:].tensor,
                            offset=sing[:].offset,
                            ap=[sing[:].ap[0], [0, HL], [32, 2], [1, 32]],
                        )
                        nc.vector.tensor_tensor(
                            out=tmp[:].rearrange("p (h a d) -> p h a d", a=2, d=32),
                            in0=swapped,
                            in1=sing_b,
                            op=ALU.mult,
                        )
                        cosg_b = bass.AP(
                            tensor=cosg[:].tensor,
                            offset=cosg[:].offset,
                            ap=[cosg[:].ap[0], [0, HL], [1, D]],
                        )
                        nc.vector.tensor_tensor(
                            out=dst[:].rearrange("p (h d) -> p h d", h=HL),
                            in0=src[:].rearrange("p (h d) -> p h d", h=HL),
                            in1=cosg_b,
                            op=ALU.mult,
                        )
                        nc.vector.tensor_tensor(out=dst[:], in0=dst[:], in1=tmp[:], op=ALU.add)

                    # transpose q into ring slot g%2, k into kT tile
                    kT = kvp.tile([128, HT, 128], F32R, tag="kT")
                    for ht in range(HT):
                        tp = ps_tr.tile([128, 128], F32, tag="tp")
                        nc.tensor.transpose(tp[:], qr[:, ht * 128 : (ht + 1) * 128], idn_t[:].bitcast(F32))
                        nc.scalar.copy(qring[:, ht, g % 2, :], tp[:])
                        tp2 = ps_tr.tile([128, 128], F32, tag="tp")
                        nc.tensor.transpose(tp2[:], kr[:, ht * 128 : (ht + 1) * 128], idn_t[:].bitcast(F32))
                        nc.scalar.copy(kT[:, ht, :], tp2[:])
                    kT_hist[g] = kT

                # ---------------- stage B: scores for key block kb=g-1 ----------------
                if g >= 1:
                    kb = g - 1
                    kTb = kT_hist.pop(kb)
                    tail = kb == NB - 1
                    colsl = slice(128, 256) if tail else slice(0, 256)
                    ncols = 128 if tail else 256
                    maskt = bb_t if (kb % 2 == 0) else bbs_t
                    pt_t = ptp.tile([128, HL, 256], F32R, tag="pt_t")
                    for hp in range(HL // 2):
                        st = ps_st.tile([128, 2, 256], F32, tag="st")
                        for j in range(2):
                            h = hp * 2 + j
                            ht, r0 = h // 2, (h % 2) * 64
                            if tail:
                                qrhs = qring[r0 : r0 + 64, ht, 1, :]
                            else:
                                qrhs = qring[r0 : r0 + 64, ht, :, :]
                            nc.tensor.matmul(
                                st[:, j, colsl],
                                kTb[r0 : r0 + 64, ht, :],
                                qrhs,
                                start=True, stop=(maskmode != "mm"),
                            )
                            if maskmode == "mm":
                                nc.tensor.matmul(
                                    st[:, j, colsl],
                                    idn_t[:],
                                    bb_t[:, 0:128] if tail else maskt[:],
                                    start=False, stop=True,
                                )
                        if maskmode == "dve2":
                            scr = ptp.tile([128, 2, 256], F32, tag="scr")
                            nc.scalar.activation(scr[:, :, colsl], st[:, :, colsl], ACTF.Exp)
                            msk = bb_t[:, 0:128] if tail else maskt[:]
                            for j in range(2):
                                nc.vector.tensor_mul(
                                    pt_t[:, hp * 2 + j, colsl],
                                    scr[:, j, colsl],
                                    msk.bitcast(F32),
                                )
                        else:
                            nc.scalar.activation(
                                pt_t[:, hp * 2 : hp * 2 + 2, colsl],
                                st[:, :, colsl],
                                ACTF.Exp,
                            )
                        if maskmode == "dve":
                            msk = bb_t[:, 0:128] if tail else maskt[:]
                            for j in range(2):
                                nc.vector.tensor_mul(
                                    pt_t[:, hp * 2 + j, colsl],
                                    pt_t[:, hp * 2 + j, colsl],
                                    msk,
                                )
                        elif maskmode == "gps":
                            msk = bb_t[:, 0:128] if tail else maskt[:]
                            for j in range(2):
                                nc.gpsimd.tensor_mul(
                                    pt_t[:, hp * 2 + j, colsl],
                                    pt_t[:, hp * 2 + j, colsl],
                                    msk,
                                )
                    pt_hist[kb] = pt_t
                    if kb >= 2:
                        pt_hist.pop(kb - 2)
                        va_hist.pop(kb - 2)

                # ---------------- stage C+D: attention out + y for qb=g-1 ----------------
                if g >= 1:
                    qb = g - 1
                    sl = (qb % 2) * 128
                    ao = att.tile([128, HL, D], F32, tag="ao")
                    rec = att.tile([128, HL], F32, tag="rec")
                    for hq in range(2):  # head quads
                        o_ps = ps_oy.tile([128, 4, 66], F32, tag="oy_ps", name="o_ps")
                        for hh in range(4):
                            h = hq * 4 + hh
                            if qb >= 1:
                                nc.tensor.matmul(
                                    o_ps[:, hh, :],
                                    pt_hist[qb - 1][:, h, sl : sl + 128],
                                    va_hist[qb - 1][:, h, :],
                                    start=True, stop=False,
                                )
                            nc.tensor.matmul(
                                o_ps[:, hh, :],
                                pt_hist[qb][:, h, sl : sl + 128],
                                va_hist[qb][:, h, :],
                                start=(qb == 0), stop=True,
                            )
                        hsl = slice(hq * 4, hq * 4 + 4)
                        nc.vector.reciprocal(
                            out=rec[:, hsl],
                            in_=o_ps[:, :, 64:65].rearrange("p h o -> p (h o)"),
                        )
                        nc.vector.tensor_tensor(
                            out=ao[:, hsl, :],
                            in0=o_ps[:, :, 0:64],
                            in1=rec[:, hsl].broadcast_to([128, 4, D]),
                            op=ALU.mult,
                        )

                    # transpose ao and project
                    aoT = att.tile([128, HT, 128], F32R, tag="aoT")
                    aov = ao[:].rearrange("p h d -> p (h d)")
                    for ht in range(HT):
                        tp3 = ps_tr.tile([128, 128], F32, tag="tp")
                        nc.tensor.transpose(tp3[:], aov[:, ht * 128 : (ht + 1) * 128], idn_t[:].bitcast(F32))
                        nc.scalar.copy(aoT[:, ht, :], tp3[:])

                    y_sb = io.tile([128, CIN], F32, tag="y_sb")
                    for half in range(2):
                        y_ps = ps_oy.tile([128, 512], F32, tag="oy_ps", name="y_ps")
                        for ct in range(HT):
                            nc.tensor.matmul(
                                y_ps[:],
                                aoT[:, ct, :],
                                wp_t[:, ct, half * 512 : (half + 1) * 512],
                                start=(ct == 0), stop=(ct == HT - 1),
                            )
                        nc.scalar.copy(y_sb[:, half * 512 : (half + 1) * 512], y_ps[:])
                    nc.sync.dma_start(y_d[qb * 128 : (qb + 1) * 128, :], y_sb[:])

    nc.compile()
    return nc


def _host_prep(x, v1, cos, sin, Wq, Wk, Wv, Wproj, lamb, maskmode="mm"):
    B = x.shape[0]
    # chronological band mask: rows=key pos in block kb, cols=256 queries
    tk = np.arange(128)[:, None]
    tq = np.arange(256)[None, :]
    valid = np.where(tq < 128, tq >= tk, (tq - 128) < tk)
    if maskmode == "mm":
        bandb = np.where(valid, 0.0, -1e30).astype(np.float32)
    else:
        bandb = valid.astype(np.float32)
    bandbsw = np.concatenate([bandb[:, 128:], bandb[:, :128]], axis=1)
    idn = np.eye(128, dtype=np.float32)

    in_maps = []
    for core in range(8):
        b, hg = core // 2, core % 2
        rsl = slice(hg * 512, (hg + 1) * 512)
        xT = np.ascontiguousarray(x[b].T)
        wqT = np.ascontiguousarray(Wq[rsl, :].T)
        wkT = np.ascontiguousarray(Wk[rsl, :].T)
        wvT = np.ascontiguousarray((1.0 - lamb) * Wv[rsl, :].T)
        wpT = np.ascontiguousarray(Wproj[:, rsl].T)
        v1s = np.ascontiguousarray(lamb * v1[b].reshape(T, 1024)[:, rsl])
        sinb = sin[b].copy()
        sinb[:, :32] *= -1.0
        in_maps.append({
            "xT": xT, "wqT": wqT, "wkT": wkT, "wvT": wvT, "wpT": wpT,
            "v1s": v1s, "cosb": np.ascontiguousarray(cos[b]),
            "sinb": np.ascontiguousarray(sinb),
            "bandb": bandb, "bandbsw": bandbsw, "idn": idn,
        })
    return in_maps


def kernel(x, v1, cos, sin, Wq, Wk, Wv, Wproj, lamb, max_seq_length=None, **_):
    x = np.asarray(x, dtype=np.float32)
    v1 = np.asarray(v1, dtype=np.float32)
    cos = np.asarray(cos, dtype=np.float32)
    sin = np.asarray(sin, dtype=np.float32)
    Wq = np.asarray(Wq, dtype=np.float32)
    Wk = np.asarray(Wk, dtype=np.float32)
    Wv = np.asarray(Wv, dtype=np.float32)
    Wproj = np.asarray(Wproj, dtype=np.float32)
    lamb = float(np.asarray(lamb))

    if "nc" not in _CACHE:
        _CACHE["nc"] = _build()
    nc = _CACHE["nc"]

    in_maps = _host_prep(x, v1, cos, sin, Wq, Wk, Wv, Wproj, lamb)
    res = run_bass_kernel_spmd(nc, in_maps, core_ids=list(range(8)))

    B = x.shape[0]
    y = np.empty((B, T, CIN), dtype=np.float32)
    for b in range(B):
        y[b] = res.results[2 * b]["y"] + res.results[2 * b + 1]["y"]
    return y, v1


# revision 15
# speedup vs baseline: 1.1562x; 1.0183x over previous
"""Trainium2 Bass kernel for nn_CausalSelfAttention_60224031424653.

Reference computation (B=4, T=4096, C=1024, H=16, D=64, sliding window 128):
  q,k,v = x@Wq.T, x@Wk.T, x@Wv.T ; v = (1-lamb)*v + lamb*v1
  q,k = rms_norm(q), rms_norm(k) (per head, over D)
  q,k = rope(q,k; cos,sin)
  out = sliding-window causal attention (each query sees previous 128 keys)
  y = out@Wproj.T ;  returns (y, v1)

Sharding (8 cores): core c = (batch b = c//2, head-group hg = c%2 of 8 heads).
Each core computes a partial y over its 512 channels' contribution to the
output projection; host sums the two partials per batch.

Per-core kernel: stream T in 128-token blocks g=0..31 with a one-block lag:
  iter g:  A) project block g -> q,k,v (fp32r matmuls, xT stationary);
              rms-norm via DVE Newton rsqrt; rope; PE-transpose q,k
           B) scores ST_{g-1} [keys of block g-1 x 256 queries] + additive
              band mask via identity matmul; exp on ACT -> PT (fp32r)
           C) attention output for query block g-1 from PT_{g-1}, PT_{g-2}
              with a fused ones-column denominator in v_aug; normalize on DVE
           D) output projection of block g-1; DMA partial y out.

fp32r (fp32 rounded to ~13 mantissa bits, exact fp32 PSUM accumulation) is
used for all matmul operands: ~4x faster than plain fp32 on the PE.
"""
import sys

sys.path.insert(0, "/opt/trn_rl_repo")

import numpy as np
import concourse.bass as bass
import concourse.mybir as mybir
import concourse.tile as tile
from concourse import bacc
from concourse.bass_utils import run_bass_kernel_spmd

F32 = mybir.dt.float32
F32R = mybir.dt.float32r
I32 = mybir.dt.int32
ALU = mybir.AluOpType
ACTF = mybir.ActivationFunctionType

T = 4096
CIN = 1024
HL = 8          # local heads per core
D = 64
COUT = HL * D   # 512
NB = T // 128   # 32 blocks
CT = CIN // 128  # 8 cin tiles
HT = COUT // 128  # 4 local cout tiles
EPS64 = 64.0 * 1e-6
MAGIC = 0x5F3759DF

_CACHE = {}


def _build(maskmode="mm"):
    nc = bacc.Bacc("TRN2", target_bir_lowering=False, debug=False, num_devices=8)

    xT_d = nc.dram_tensor("xT", [CIN, T], F32R, kind="ExternalInput").ap()
    wq_d = nc.dram_tensor("wqT", [CIN, COUT], F32R, kind="ExternalInput").ap()
    wk_d = nc.dram_tensor("wkT", [CIN, COUT], F32R, kind="ExternalInput").ap()
    wv_d = nc.dram_tensor("wvT", [CIN, COUT], F32R, kind="ExternalInput").ap()
    wp_d = nc.dram_tensor("wpT", [COUT, CIN], F32R, kind="ExternalInput").ap()
    v1_d = nc.dram_tensor("v1s", [T, COUT], F32, kind="ExternalInput").ap()
    cos_d = nc.dram_tensor("cosb", [T, D], F32, kind="ExternalInput").ap()
    sin_d = nc.dram_tensor("sinb", [T, D], F32, kind="ExternalInput").ap()
    bb_d = nc.dram_tensor("bandb", [128, 256], F32R, kind="ExternalInput").ap()
    bbs_d = nc.dram_tensor("bandbsw", [128, 256], F32R, kind="ExternalInput").ap()
    idn_d = nc.dram_tensor("idn", [128, 128], F32R, kind="ExternalInput").ap()
    y_d = nc.dram_tensor("y", [T, CIN], F32, kind="ExternalOutput").ap()

    xT_v = xT_d.rearrange("(ct p) t -> p ct t", p=128)
    wq_v = wq_d.rearrange("(ct p) n -> p ct n", p=128)
    wk_v = wk_d.rearrange("(ct p) n -> p ct n", p=128)
    wv_v = wv_d.rearrange("(ct p) n -> p ct n", p=128)
    wp_v = wp_d.rearrange("(ct p) n -> p ct n", p=128)

    with tile.TileContext(nc) as tc:
        with (
            tc.tile_pool(name="wpool", bufs=1) as wpool,
            tc.tile_pool(name="ring", bufs=1) as ringp,
            tc.tile_pool(name="io", bufs=3) as io,
            tc.tile_pool(name="stg", bufs=2) as stg,
            tc.tile_pool(name="kv", bufs=3) as kvp,
            tc.tile_pool(name="vau", bufs=4) as vaup,
            tc.tile_pool(name="pt", bufs=3) as ptp,
            tc.tile_pool(name="attn", bufs=2) as att,
            tc.tile_pool(name="ps_qkv", bufs=2, space="PSUM") as ps_qkv,
            tc.tile_pool(name="ps_tr", bufs=2, space="PSUM") as ps_tr,
            tc.tile_pool(name="ps_st", bufs=2, space="PSUM") as ps_st,
            tc.tile_pool(name="ps_oy", bufs=2, space="PSUM") as ps_oy,
        ):
            wq_t = wpool.tile([128, CT, COUT], F32R, name="wq_t")
            wk_t = wpool.tile([128, CT, COUT], F32R, name="wk_t")
            wv_t = wpool.tile([128, CT, COUT], F32R, name="wv_t")
            wp_t = wpool.tile([128, HT, CIN], F32R, name="wp_t")
            bb_t = wpool.tile([128, 256], F32R, name="bb_t")
            bbs_t = wpool.tile([128, 256], F32R, name="bbs_t")
            idn_t = wpool.tile([128, 128], F32R, name="idn_t")
            nc.sync.dma_start(wq_t[:], wq_v)
            nc.sync.dma_start(wk_t[:], wk_v)
            nc.sync.dma_start(wv_t[:], wv_v)
            nc.sync.dma_start(wp_t[:], wp_v)
            nc.sync.dma_start(bb_t[:], bb_d)
            nc.sync.dma_start(bbs_t[:], bbs_d)
            nc.sync.dma_start(idn_t[:], idn_d)

            # qT ring: [d-of-2-heads(128), ht, slot, t]
            qring = ringp.tile([128, HT, 2, 128], F32R, name="qring")

            kT_hist = {}    # g -> kT tile [128, HT, 128]
            va_hist = {}    # g -> v_aug tile [128, HL, 66]
            pt_hist = {}    # g -> PT tile [128, HL, 256]

            for g in range(NB + 1):
                # ---------------- stage A: project block g ----------------
                if g < NB:
                    c0 = g * 128
                    xg = io.tile([128, CT, 128], F32R, tag="xg")
                    nc.sync.dma_start(xg[:], xT_v[:, :, c0 : c0 + 128])
                    v1g = io.tile([128, COUT], F32, tag="v1g")
                    nc.sync.dma_start(v1g[:], v1_d[c0 : c0 + 128, :])
                    cosg = io.tile([128, D], F32, tag="cosg")
                    nc.sync.dma_start(cosg[:], cos_d[c0 : c0 + 128, :])
                    sing = io.tile([128, D], F32, tag="sing")
                    nc.sync.dma_start(sing[:], sin_d[c0 : c0 + 128, :])

                    q_ps = ps_qkv.tile([128, COUT], F32, tag="qkv_ps", name="q_ps")
                    k_ps = ps_qkv.tile([128, COUT], F32, tag="qkv_ps", name="k_ps")
                    v_ps = ps_qkv.tile([128, COUT], F32, tag="qkv_ps", name="v_ps")
                    for ct in range(CT):
                        se = dict(start=(ct == 0), stop=(ct == CT - 1))
                        nc.tensor.matmul(q_ps[:], xg[:, ct, :], wq_t[:, ct, :], **se)
                    for ct in range(CT):
                        se = dict(start=(ct == 0), stop=(ct == CT - 1))
                        nc.tensor.matmul(k_ps[:], xg[:, ct, :], wk_t[:, ct, :], **se)
                    for ct in range(CT):
                        se = dict(start=(ct == 0), stop=(ct == CT - 1))
                        nc.tensor.matmul(v_ps[:], xg[:, ct, :], wv_t[:, ct, :], **se)

                    # v_aug: [t, h, 0:64]=v + v1s (pre-scaled), [t, h, 64:66]=1
                    v_aug = vaup.tile([128, HL, 66], F32R, tag="v_aug")
                    nc.vector.memset(v_aug[:, :, 64:66].bitcast(F32), 1.0)
                    nc.vector.tensor_tensor(
                        out=v_aug[:, :, 0:64],
                        in0=v_ps[:].rearrange("p (h d) -> p h d", h=HL),
                        in1=v1g[:].rearrange("p (h d) -> p h d", h=HL),
                        op=ALU.add,
                    )
                    va_hist[g] = v_aug

                    # copy q,k to SBUF so the PSUM banks free quickly
                    qsb = stg.tile([128, COUT], F32, tag="qsb")
                    ksb = stg.tile([128, COUT], F32, tag="ksb")
                    nc.scalar.copy(qsb[:], q_ps[:])
                    nc.scalar.copy(ksb[:], k_ps[:])
                    # sum of squares for q (cols 0:8) and k (cols 8:16)
                    sq = stg.tile([128, COUT], F32, tag="sq")
                    ssq = stg.tile([128, 2 * HL], F32, tag="ssq")
                    nc.vector.tensor_tensor(out=sq[:], in0=qsb[:], in1=qsb[:], op=ALU.mult)
                    nc.vector.reduce_sum(
                        out=ssq[:, 0:HL],
                        in_=sq[:].rearrange("p (h d) -> p h d", h=HL),
                        axis=mybir.AxisListType.X,
                    )
                    nc.vector.tensor_tensor(out=sq[:], in0=ksb[:], in1=ksb[:], op=ALU.mult)
                    nc.vector.reduce_sum(
                        out=ssq[:, HL : 2 * HL],
                        in_=sq[:].rearrange("p (h d) -> p h d", h=HL),
                        axis=mybir.AxisListType.X,
                    )
                    nc.vector.tensor_scalar_add(ssq[:], in0=ssq[:], scalar1=EPS64)
                    # Newton rsqrt: r = 1/sqrt(ssq); q scale=r, k scale=8r
                    rsy = stg.tile([128, 2 * HL], F32, tag="rsy")
                    rst = stg.tile([128, 2 * HL], F32, tag="rst")
                    nc.vector.tensor_scalar(
                        out=rst[:].bitcast(I32),
                        in0=ssq[:].bitcast(I32),
                        scalar1=1,
                        scalar2=None,
                        op0=ALU.logical_shift_right,
                    )
                    nc.vector.tensor_scalar(
                        out=rsy[:].bitcast(I32),
                        in0=rst[:].bitcast(I32),
                        scalar1=-1,
                        scalar2=None,
                        op0=ALU.bitwise_xor,
                    )
                    nc.vector.tensor_scalar(
                        out=rsy[:].bitcast(I32),
                        in0=rsy[:].bitcast(I32),
                        scalar1=MAGIC + 1,
                        scalar2=None,
                        op0=ALU.add,
                    )
                    for _ in range(3):
                        nc.vector.tensor_tensor(out=rst[:], in0=ssq[:], in1=rsy[:], op=ALU.mult)
                        nc.vector.tensor_tensor(out=rst[:], in0=rst[:], in1=rsy[:], op=ALU.mult)
                        nc.vector.tensor_scalar(
                            out=rst[:], in0=rst[:], scalar1=-0.5, scalar2=1.5,
                            op0=ALU.mult, op1=ALU.add,
                        )
                        nc.vector.tensor_tensor(out=rsy[:], in0=rsy[:], in1=rst[:], op=ALU.mult)

                    # normalize (q also folds the 1/8 attention scale)
                    qn = stg.tile([128, COUT], F32, tag="qn")
                    kn = stg.tile([128, COUT], F32, tag="kn")
                    nc.vector.tensor_tensor(
                        out=qn[:].rearrange("p (h d) -> p h d", h=HL),
                        in0=qsb[:].rearrange("p (h d) -> p h d", h=HL),
                        in1=rsy[:, 0:HL].broadcast_to([128, HL, D]),
                        op=ALU.mult,
                    )
                    nc.vector.scalar_tensor_tensor(
                        out=kn[:].rearrange("p (h d) -> p h d", h=HL),
                        in0=ksb[:].rearrange("p (h d) -> p h d", h=HL),
                        scalar=8.0,
                        in1=rsy[:, HL : 2 * HL].broadcast_to([128, HL, D]),
                        op0=ALU.mult,
                        op1=ALU.mult,
                    )

                    # rope: out = qn*cos + swap_halves(qn)*sin_signed
                    qr = stg.tile([128, COUT], F32, tag="qr")
                    kr = stg.tile([128, COUT], F32, tag="kr")
                    tmp = stg.tile([128, COUT], F32, tag="tmp")
                    for (src, dst) in ((qn, qr), (kn, kr)):
                        sv = src[:].rearrange("p (h a d) -> p h a d", a=2, d=32)
                        swapped = bass.AP(
                            tensor=sv.tensor,
                            offset=sv.offset + 32,
                            ap=[sv.ap[0], sv.ap[1], [-32, 2], [1, 32]],
                        )
                        sing_b = bass.AP(
                            tensor=sing[:].tensor,
                            offset=sing[:].offset,
                            ap=[sing[:].ap[0], [0, HL], [32, 2], [1, 32]],
                        )
                        nc.vector.tensor_tensor(
                            out=tmp[:].rearrange("p (h a d) -> p h a d", a=2, d=32),
                            in0=swapped,
                            in1=sing_b,
                            op=ALU.mult,
                        )
                        cosg_b = bass.AP(
                            tensor=cosg[:].tensor,
                            offset=cosg[:].offset,
                            ap=[cosg[:].ap[0], [0, HL], [1, D]],
                        )
                        nc.vector.tensor_tensor(
                            out=dst[:].rearrange("p (h d) -> p h d", h=HL),
                            in0=src[:].rearrange("p (h d) -> p h d", h=HL),
                            in1=cosg_b,
                            op=ALU.mult,
                        )
                        nc.vector.tensor_tensor(out=dst[:], in0=dst[:], in1=tmp[:], op=ALU.add)

                    # transpose q into ring slot g%2, k into kT tile
                    kT = kvp.tile([128, HT, 128], F32R, tag="kT")
                    for ht in range(HT):
                        tp = ps_tr.tile([128, 128], F32, tag="tp")
                        nc.tensor.transpose(tp[:], qr[:, ht * 128 : (ht + 1) * 128], idn_t[:].bitcast(F32))
                        nc.scalar.copy(qring[:, ht, g % 2, :], tp[:])
                        tp2 = ps_tr.tile([128, 128], F32, tag="tp")
                        nc.tensor.transpose(tp2[:], kr[:, ht * 128 : (ht + 1) * 128], idn_t[:].bitcast(F32))
                        nc.scalar.copy(kT[:, ht, :], tp2[:])
                    kT_hist[g] = kT

                # ---------------- stage B: scores for key block kb=g-1 ----------------
                if g >= 1:
                    kb = g - 1
                    kTb = kT_hist.pop(kb)
                    tail = kb == NB - 1
                    colsl = slice(128, 256) if tail else slice(0, 256)
                    ncols = 128 if tail else 256
                    maskt = bb_t if (kb % 2 == 0) else bbs_t
                    pt_t = ptp.tile([128, HL, 256], F32R, tag="pt_t")
                    for hp in range(HL // 2):
                        st = ps_st.tile([128, 2, 256], F32, tag="st")
                        for j in range(2):
                            h = hp * 2 + j
                            ht, r0 = h // 2, (h % 2) * 64
                            if tail:
                                qrhs = qring[r0 : r0 + 64, ht, 1, :]
                            else:
                                qrhs = qring[r0 : r0 + 64, ht, :, :]
                            nc.tensor.matmul(
                                st[:, j, colsl],
                                kTb[r0 : r0 + 64, ht, :],
                                qrhs,
                                start=True, stop=(maskmode != "mm"),
                            )
                            if maskmode == "mm":
                                nc.tensor.matmul(
                                    st[:, j, colsl],
                                    idn_t[:],
                                    bb_t[:, 0:128] if tail else maskt[:],
                                    start=False, stop=True,
                                )
                        if maskmode == "dve2":
                            scr = ptp.tile([128, 2, 256], F32, tag="scr")
                            nc.scalar.activation(scr[:, :, colsl], st[:, :, colsl], ACTF.Exp)
                            msk = bb_t[:, 0:128] if tail else maskt[:]
                            for j in range(2):
                                nc.vector.tensor_mul(
                                    pt_t[:, hp * 2 + j, colsl],
                                    scr[:, j, colsl],
                                    msk.bitcast(F32),
                                )
                        else:
                            nc.scalar.activation(
                                pt_t[:, hp * 2 : hp * 2 + 2, colsl],
                                st[:, :, colsl],
                                ACTF.Exp,
                            )
                        if maskmode == "dve":
                            msk = bb_t[:, 0:128] if tail else maskt[:]
                            for j in range(2):
                                nc.vector.tensor_mul(
                                    pt_t[:, hp * 2 + j, colsl],
                                    pt_t[:, hp * 2 + j, colsl],
                                    msk,
                                )
                        elif maskmode == "gps":
                            msk = bb_t[:, 0:128] if tail else maskt[:]
                            for j in range(2):
                                nc.gpsimd.tensor_mul(
                                    pt_t[:, hp * 2 + j, colsl],
                                    pt_t[:, hp * 2 + j, colsl],
                                    msk,
                                )
                    pt_hist[kb] = pt_t
                    if kb >= 2:
                        pt_hist.pop(kb - 2)
                        va_hist.pop(kb - 2)

                # ---------------- stage C+D: attention out + y for qb=g-1 ----------------
                if g >= 1:
                    qb = g - 1
                    sl = (qb % 2) * 128
                    ao = att.tile([128, HL, D], F32, tag="ao")
                    rec = att.tile([128, HL], F32, tag="rec")
                    for hq in range(2):  # head quads
                        o_ps = ps_oy.tile([128, 4, 66], F32, tag="oy_ps", name="o_ps")
                        for hh in range(4):
                            h = hq * 4 + hh
                            if qb >= 1:
                                nc.tensor.matmul(
                                    o_ps[:, hh, :],
                                    pt_hist[qb - 1][:, h, sl : sl + 128],
                                    va_hist[qb - 1][:, h, :],
                                    start=True, stop=False,
                                )
                            nc.tensor.matmul(
                                o_ps[:, hh, :],
                                pt_hist[qb][:, h, sl : sl + 128],
                                va_hist[qb][:, h, :],
                                start=(qb == 0), stop=True,
                            )
                        hsl = slice(hq * 4, hq * 4 + 4)
                        nc.vector.reciprocal(
                            out=rec[:, hsl],
                            in_=o_ps[:, :, 64:65].rearrange("p h o -> p (h o)"),
                        )
                        nc.vector.tensor_tensor(
                            out=ao[:, hsl, :],
                            in0=o_ps[:, :, 0:64],
                            in1=rec[:, hsl].broadcast_to([128, 4, D]),
                            op=ALU.mult,
                        )

                    # transpose ao and project
                    aoT = att.tile([128, HT, 128], F32R, tag="aoT")
                    aov = ao[:].rearrange("p h d -> p (h d)")
                    for ht in range(HT):
                        tp3 = ps_tr.tile([128, 128], F32, tag="tp")
                        nc.tensor.transpose(tp3[:], aov[:, ht * 128 : (ht + 1) * 128], idn_t[:].bitcast(F32))
                        nc.scalar.copy(aoT[:, ht, :], tp3[:])

                    y_sb = io.tile([128, CIN], F32, tag="y_sb")
                    for half in range(2):
                        y_ps = ps_oy.tile([128, 512], F32, tag="oy_ps", name="y_ps")
                        for ct in range(HT):
                            nc.tensor.matmul(
                                y_ps[:],
                                aoT[:, ct, :],
                                wp_t[:, ct, half * 512 : (half + 1) * 512],
                                start=(ct == 0), stop=(ct == HT - 1),
                            )
                        nc.scalar.copy(y_sb[:, half * 512 : (half + 1) * 512], y_ps[:])
                    nc.sync.dma_start(y_d[qb * 128 : (qb + 1) * 128, :], y_sb[:])

    nc.compile()
    return nc


def _host_prep(x, v1, cos, sin, Wq, Wk, Wv, Wproj, lamb, maskmode="mm"):
    B = x.shape[0]
    # chronological band mask: rows=key pos in block kb, cols=256 queries
    tk = np.arange(128)[:, None]
    tq = np.arange(256)[None, :]
    valid = np.where(tq < 128, tq >= tk, (tq - 128) < tk)
    if maskmode == "mm":
        bandb = np.where(valid, 0.0, -1e30).astype(np.float32)
    else:
        bandb = valid.astype(np.float32)
    bandbsw = np.concatenate([bandb[:, 128:], bandb[:, :128]], axis=1)
    idn = np.eye(128, dtype=np.float32)

    in_maps = []
    for core in range(8):
        b, hg = core // 2, core % 2
        rsl = slice(hg * 512, (hg + 1) * 512)
        xT = np.ascontiguousarray(x[b].T)
        wqT = np.ascontiguousarray(Wq[rsl, :].T)
        wkT = np.ascontiguousarray(Wk[rsl, :].T)
        wvT = np.ascontiguousarray((1.0 - lamb) * Wv[rsl, :].T)
        wpT = np.ascontiguousarray(Wproj[:, rsl].T)
        v1s = np.ascontiguousarray(lamb * v1[b].reshape(T, 1024)[:, rsl])
        sinb = sin[b].copy()
        sinb[:, :32] *= -1.0
        in_maps.append({
            "xT": xT, "wqT": wqT, "wkT": wkT, "wvT": wvT, "wpT": wpT,
            "v1s": v1s, "cosb": np.ascontiguousarray(cos[b]),
            "sinb": np.ascontiguousarray(sinb),
            "bandb": bandb, "bandbsw": bandbsw, "idn": idn,
        })
    return in_maps


def kernel(x, v1, cos, sin, Wq, Wk, Wv, Wproj, lamb, max_seq_length=None, **_):
    x = np.asarray(x, dtype=np.float32)
    v1 = np.asarray(v1, dtype=np.float32)
    cos = np.asarray(cos, dtype=np.float32)
    sin = np.asarray(sin, dtype=np.float32)
    Wq = np.asarray(Wq, dtype=np.float32)
    Wk = np.asarray(Wk, dtype=np.float32)
    Wv = np.asarray(Wv, dtype=np.float32)
    Wproj = np.asarray(Wproj, dtype=np.float32)
    lamb = float(np.asarray(lamb))

    if "nc" not in _CACHE:
        _CACHE["nc"] = _build()
    nc = _CACHE["nc"]

    in_maps = _host_prep(x, v1, cos, sin, Wq, Wk, Wv, Wproj, lamb)
    res = run_bass_kernel_spmd(nc, in_maps, core_ids=list(range(8)))

    B = x.shape[0]
    y = np.empty((B, T, CIN), dtype=np.float32)
    for b in range(B):
        y[b] = res.results[2 * b]["y"] + res.results[2 * b + 1]["y"]
    return y, v1


# revision 18
# speedup vs baseline: 1.1737x; 1.0152x over previous
"""Trainium2 Bass kernel for nn_CausalSelfAttention_60224031424653.

Reference computation (B=4, T=4096, C=1024, H=16, D=64, sliding window 128):
  q,k,v = x@Wq.T, x@Wk.T, x@Wv.T ; v = (1-lamb)*v + lamb*v1
  q,k = rms_norm(q), rms_norm(k) (per head, over D)
  q,k = rope(q,k; cos,sin)
  out = sliding-window causal attention (each query sees previous 128 keys)
  y = out@Wproj.T ;  returns (y, v1)

Sharding (8 cores): core c = (batch b = c//2, head-group hg = c%2 of 8 heads).
Each core computes a partial y over its 512 channels' contribution to the
output projection; host sums the two partials per batch.

Per-core kernel: stream T in 128-token blocks g=0..31 with a one-block lag:
  iter g:  A) project block g -> q,k,v (fp32r matmuls, xT stationary);
              rms-norm via DVE Newton rsqrt; rope; PE-transpose q,k
           B) scores ST_{g-1} [keys of block g-1 x 256 queries] + additive
              band mask via identity matmul; exp on ACT -> PT (fp32r)
           C) attention output for query block g-1 from PT_{g-1}, PT_{g-2}
              with a fused ones-column denominator in v_aug; normalize on DVE
           D) output projection of block g-1; DMA partial y out.

fp32r (fp32 rounded to ~13 mantissa bits, exact fp32 PSUM accumulation) is
used for all matmul operands: ~4x faster than plain fp32 on the PE.
"""
import sys

sys.path.insert(0, "/opt/trn_rl_repo")

import numpy as np
import concourse.bass as bass
import concourse.mybir as mybir
import concourse.tile as tile
from concourse import bacc
from concourse.bass_utils import run_bass_kernel_spmd

F32 = mybir.dt.float32
F32R = mybir.dt.float32r
I32 = mybir.dt.int32
ALU = mybir.AluOpType
ACTF = mybir.ActivationFunctionType

T = 4096
CIN = 1024
HL = 8          # local heads per core
D = 64
COUT = HL * D   # 512
NB = T // 128   # 32 blocks
CT = CIN // 128  # 8 cin tiles
HT = COUT // 128  # 4 local cout tiles
EPS64 = 64.0 * 1e-6
MAGIC = 0x5F3759DF

_CACHE = {}


def _build(maskmode="mm"):
    nc = bacc.Bacc("TRN2", target_bir_lowering=False, debug=False, num_devices=8)

    xT_d = nc.dram_tensor("xT", [CIN, T], F32R, kind="ExternalInput").ap()
    wq_d = nc.dram_tensor("wqT", [CIN, COUT], F32R, kind="ExternalInput").ap()
    wk_d = nc.dram_tensor("wkT", [CIN, COUT], F32R, kind="ExternalInput").ap()
    wv_d = nc.dram_tensor("wvT", [CIN, COUT], F32R, kind="ExternalInput").ap()
    wp_d = nc.dram_tensor("wpT", [COUT, CIN], F32R, kind="ExternalInput").ap()
    v1_d = nc.dram_tensor("v1s", [T, COUT], F32, kind="ExternalInput").ap()
    cos_d = nc.dram_tensor("cosb", [T, D], F32, kind="ExternalInput").ap()
    sin_d = nc.dram_tensor("sinb", [T, D], F32, kind="ExternalInput").ap()
    bb_d = nc.dram_tensor("bandb", [128, 256], F32R, kind="ExternalInput").ap()
    bbs_d = nc.dram_tensor("bandbsw", [128, 256], F32R, kind="ExternalInput").ap()
    idn_d = nc.dram_tensor("idn", [128, 128], F32R, kind="ExternalInput").ap()
    y_d = nc.dram_tensor("y", [T, CIN], F32, kind="ExternalOutput").ap()

    xT_v = xT_d.rearrange("(ct p) t -> p ct t", p=128)
    wq_v = wq_d.rearrange("(ct p) n -> p ct n", p=128)
    wk_v = wk_d.rearrange("(ct p) n -> p ct n", p=128)
    wv_v = wv_d.rearrange("(ct p) n -> p ct n", p=128)
    wp_v = wp_d.rearrange("(ct p) n -> p ct n", p=128)

    with tile.TileContext(nc) as tc:
        with (
            tc.tile_pool(name="wpool", bufs=1) as wpool,
            tc.tile_pool(name="ring", bufs=1) as ringp,
            tc.tile_pool(name="io", bufs=3) as io,
            tc.tile_pool(name="stg", bufs=3) as stg,
            tc.tile_pool(name="kv", bufs=3) as kvp,
            tc.tile_pool(name="vau", bufs=4) as vaup,
            tc.tile_pool(name="pt", bufs=3) as ptp,
            tc.tile_pool(name="attn", bufs=3) as att,
            tc.tile_pool(name="ps_qkv", bufs=2, space="PSUM") as ps_qkv,
            tc.tile_pool(name="ps_tr", bufs=2, space="PSUM") as ps_tr,
            tc.tile_pool(name="ps_st", bufs=2, space="PSUM") as ps_st,
            tc.tile_pool(name="ps_oy", bufs=2, space="PSUM") as ps_oy,
        ):
            wq_t = wpool.tile([128, CT, COUT], F32R, name="wq_t")
            wk_t = wpool.tile([128, CT, COUT], F32R, name="wk_t")
            wv_t = wpool.tile([128, CT, COUT], F32R, name="wv_t")
            wp_t = wpool.tile([128, HT, CIN], F32R, name="wp_t")
            bb_t = wpool.tile([128, 256], F32R, name="bb_t")
            bbs_t = wpool.tile([128, 256], F32R, name="bbs_t")
            idn_t = wpool.tile([128, 128], F32R, name="idn_t")
            nc.sync.dma_start(wq_t[:], wq_v)
            nc.sync.dma_start(wk_t[:], wk_v)
            nc.sync.dma_start(wv_t[:], wv_v)
            nc.sync.dma_start(wp_t[:], wp_v)
            nc.sync.dma_start(bb_t[:], bb_d)
            nc.sync.dma_start(bbs_t[:], bbs_d)
            nc.sync.dma_start(idn_t[:], idn_d)

            # qT ring: [d-of-2-heads(128), ht, slot, t]
            qring = ringp.tile([128, HT, 2, 128], F32R, name="qring")

            kT_hist = {}    # g -> kT tile [128, HT, 128]
            va_hist = {}    # g -> v_aug tile [128, HL, 66]
            pt_hist = {}    # g -> PT tile [128, HL, 256]

            for g in range(NB + 1):
                # ---------------- stage A: project block g ----------------
                if g < NB:
                    c0 = g * 128
                    xg = io.tile([128, CT, 128], F32R, tag="xg")
                    nc.sync.dma_start(xg[:], xT_v[:, :, c0 : c0 + 128])
                    v1g = io.tile([128, COUT], F32, tag="v1g")
                    nc.sync.dma_start(v1g[:], v1_d[c0 : c0 + 128, :])
                    cosg = io.tile([128, D], F32, tag="cosg")
                    nc.sync.dma_start(cosg[:], cos_d[c0 : c0 + 128, :])
                    sing = io.tile([128, D], F32, tag="sing")
                    nc.sync.dma_start(sing[:], sin_d[c0 : c0 + 128, :])

                    q_ps = ps_qkv.tile([128, COUT], F32, tag="qkv_ps", name="q_ps")
                    k_ps = ps_qkv.tile([128, COUT], F32, tag="qkv_ps", name="k_ps")
                    v_ps = ps_qkv.tile([128, COUT], F32, tag="qkv_ps", name="v_ps")
                    for ct in range(CT):
                        se = dict(start=(ct == 0), stop=(ct == CT - 1))
                        nc.tensor.matmul(q_ps[:], xg[:, ct, :], wq_t[:, ct, :], **se)
                    for ct in range(CT):
                        se = dict(start=(ct == 0), stop=(ct == CT - 1))
                        nc.tensor.matmul(k_ps[:], xg[:, ct, :], wk_t[:, ct, :], **se)
                    for ct in range(CT):
                        se = dict(start=(ct == 0), stop=(ct == CT - 1))
                        nc.tensor.matmul(v_ps[:], xg[:, ct, :], wv_t[:, ct, :], **se)

                    # v_aug: [t, h, 0:64]=v + v1s (pre-scaled), [t, h, 64:66]=1
                    v_aug = vaup.tile([128, HL, 66], F32R, tag="v_aug")
                    nc.vector.memset(v_aug[:, :, 64:66].bitcast(F32), 1.0)
                    nc.vector.tensor_tensor(
                        out=v_aug[:, :, 0:64],
                        in0=v_ps[:].rearrange("p (h d) -> p h d", h=HL),
                        in1=v1g[:].rearrange("p (h d) -> p h d", h=HL),
                        op=ALU.add,
                    )
                    va_hist[g] = v_aug

                    # copy q,k to SBUF so the PSUM banks free quickly
                    qsb = stg.tile([128, COUT], F32, tag="qsb")
                    ksb = stg.tile([128, COUT], F32, tag="ksb")
                    nc.scalar.copy(qsb[:], q_ps[:])
                    nc.scalar.copy(ksb[:], k_ps[:])
                    # sum of squares for q (cols 0:8) and k (cols 8:16)
                    sq = stg.tile([128, COUT], F32, tag="sq")
                    ssq = stg.tile([128, 2 * HL], F32, tag="ssq")
                    nc.vector.tensor_tensor(out=sq[:], in0=qsb[:], in1=qsb[:], op=ALU.mult)
                    nc.vector.reduce_sum(
                        out=ssq[:, 0:HL],
                        in_=sq[:].rearrange("p (h d) -> p h d", h=HL),
                        axis=mybir.AxisListType.X,
                    )
                    nc.vector.tensor_tensor(out=sq[:], in0=ksb[:], in1=ksb[:], op=ALU.mult)
                    nc.vector.reduce_sum(
                        out=ssq[:, HL : 2 * HL],
                        in_=sq[:].rearrange("p (h d) -> p h d", h=HL),
                        axis=mybir.AxisListType.X,
                    )
                    nc.vector.tensor_scalar_add(ssq[:], in0=ssq[:], scalar1=EPS64)
                    # Newton rsqrt: r = 1/sqrt(ssq); q scale=r, k scale=8r
                    rsy = stg.tile([128, 2 * HL], F32, tag="rsy")
                    rst = stg.tile([128, 2 * HL], F32, tag="rst")
                    nc.vector.tensor_scalar(
                        out=rst[:].bitcast(I32),
                        in0=ssq[:].bitcast(I32),
                        scalar1=1,
                        scalar2=None,
                        op0=ALU.logical_shift_right,
                    )
                    nc.vector.tensor_scalar(
                        out=rsy[:].bitcast(I32),
                        in0=rst[:].bitcast(I32),
                        scalar1=-1,
                        scalar2=None,
                        op0=ALU.bitwise_xor,
                    )
                    nc.vector.tensor_scalar(
                        out=rsy[:].bitcast(I32),
                        in0=rsy[:].bitcast(I32),
                        scalar1=MAGIC + 1,
                        scalar2=None,
                        op0=ALU.add,
                    )
                    for _ in range(3):
                        nc.vector.tensor_tensor(out=rst[:], in0=ssq[:], in1=rsy[:], op=ALU.mult)
                        nc.vector.tensor_tensor(out=rst[:], in0=rst[:], in1=rsy[:], op=ALU.mult)
                        nc.vector.tensor_scalar(
                            out=rst[:], in0=rst[:], scalar1=-0.5, scalar2=1.5,
                            op0=ALU.mult, op1=ALU.add,
                        )
                        nc.vector.tensor_tensor(out=rsy[:], in0=rsy[:], in1=rst[:], op=ALU.mult)

                    # normalize (q also folds the 1/8 attention scale)
                    qn = stg.tile([128, COUT], F32, tag="qn")
                    kn = stg.tile([128, COUT], F32, tag="kn")
                    nc.vector.tensor_tensor(
                        out=qn[:].rearrange("p (h d) -> p h d", h=HL),
                        in0=qsb[:].rearrange("p (h d) -> p h d", h=HL),
                        in1=rsy[:, 0:HL].broadcast_to([128, HL, D]),
                        op=ALU.mult,
                    )
                    nc.vector.scalar_tensor_tensor(
                        out=kn[:].rearrange("p (h d) -> p h d", h=HL),
                        in0=ksb[:].rearrange("p (h d) -> p h d", h=HL),
                        scalar=8.0,
                        in1=rsy[:, HL : 2 * HL].broadcast_to([128, HL, D]),
                        op0=ALU.mult,
                        op1=ALU.mult,
                    )

                    # rope: out = qn*cos + swap_halves(qn)*sin_signed
                    qr = stg.tile([128, COUT], F32, tag="qr")
                    kr = stg.tile([128, COUT], F32, tag="kr")
                    tmp = stg.tile([128, COUT], F32, tag="tmp")
                    for (src, dst) in ((qn, qr), (kn, kr)):
                        sv = src[:].rearrange("p (h a d) -> p h a d", a=2, d=32)
                        swapped = bass.AP(
                            tensor=sv.tensor,
                            offset=sv.offset + 32,
                            ap=[sv.ap[0], sv.ap[1], [-32, 2], [1, 32]],
                        )
                        sing_b = bass.AP(
                            tensor=sing[:].tensor,
                            offset=sing[:].offset,
                            ap=[sing[:].ap[0], [0, HL], [32, 2], [1, 32]],
                        )
                        nc.vector.tensor_tensor(
                            out=tmp[:].rearrange("p (h a d) -> p h a d", a=2, d=32),
                            in0=swapped,
                            in1=sing_b,
                            op=ALU.mult,
                        )
                        cosg_b = bass.AP(
                            tensor=cosg[:].tensor,
                            offset=cosg[:].offset,
                            ap=[cosg[:].ap[0], [0, HL], [1, D]],
                        )
                        nc.vector.tensor_tensor(
                            out=dst[:].rearrange("p (h d) -> p h d", h=HL),
                            in0=src[:].rearrange("p (h d) -> p h d", h=HL),
                            in1=cosg_b,
                            op=ALU.mult,
                        )
                        nc.vector.tensor_tensor(out=dst[:], in0=dst[:], in1=tmp[:], op=ALU.add)

                    # transpose q into ring slot g%2, k into kT tile
                    kT = kvp.tile([128, HT, 128], F32R, tag="kT")
                    for ht in range(HT):
                        tp = ps_tr.tile([128, 128], F32, tag="tp")
                        nc.tensor.transpose(tp[:], qr[:, ht * 128 : (ht + 1) * 128], idn_t[:].bitcast(F32))
                        nc.scalar.copy(qring[:, ht, g % 2, :], tp[:])
                        tp2 = ps_tr.tile([128, 128], F32, tag="tp")
                        nc.tensor.transpose(tp2[:], kr[:, ht * 128 : (ht + 1) * 128], idn_t[:].bitcast(F32))
                        nc.scalar.copy(kT[:, ht, :], tp2[:])
                    kT_hist[g] = kT

                # ---------------- stage B: scores for key block kb=g-1 ----------------
                if g >= 1:
                    kb = g - 1
                    kTb = kT_hist.pop(kb)
                    tail = kb == NB - 1
                    colsl = slice(128, 256) if tail else slice(0, 256)
                    ncols = 128 if tail else 256
                    maskt = bb_t if (kb % 2 == 0) else bbs_t
                    pt_t = ptp.tile([128, HL, 256], F32R, tag="pt_t")
                    for hp in range(HL // 2):
                        st = ps_st.tile([128, 2, 256], F32, tag="st")
                        for j in range(2):
                            h = hp * 2 + j
                            ht, r0 = h // 2, (h % 2) * 64
                            if tail:
                                qrhs = qring[r0 : r0 + 64, ht, 1, :]
                            else:
                                qrhs = qring[r0 : r0 + 64, ht, :, :]
                            nc.tensor.matmul(
                                st[:, j, colsl],
                                kTb[r0 : r0 + 64, ht, :],
                                qrhs,
                                start=True, stop=(maskmode != "mm"),
                            )
                            if maskmode == "mm":
                                nc.tensor.matmul(
                                    st[:, j, colsl],
                                    idn_t[:],
                                    bb_t[:, 0:128] if tail else maskt[:],
                                    start=False, stop=True,
                                )
                        if maskmode == "dve2":
                            scr = ptp.tile([128, 2, 256], F32, tag="scr")
                            nc.scalar.activation(scr[:, :, colsl], st[:, :, colsl], ACTF.Exp)
                            msk = bb_t[:, 0:128] if tail else maskt[:]
                            for j in range(2):
                                nc.vector.tensor_mul(
                                    pt_t[:, hp * 2 + j, colsl],
                                    scr[:, j, colsl],
                                    msk.bitcast(F32),
                                )
                        else:
                            nc.scalar.activation(
                                pt_t[:, hp * 2 : hp * 2 + 2, colsl],
                                st[:, :, colsl],
                                ACTF.Exp,
                            )
                        if maskmode == "dve":
                            msk = bb_t[:, 0:128] if tail else maskt[:]
                            for j in range(2):
                                nc.vector.tensor_mul(
                                    pt_t[:, hp * 2 + j, colsl],
                                    pt_t[:, hp * 2 + j, colsl],
                                    msk,
                                )
                        elif maskmode == "gps":
                            msk = bb_t[:, 0:128] if tail else maskt[:]
                            for j in range(2):
                                nc.gpsimd.tensor_mul(
                                    pt_t[:, hp * 2 + j, colsl],
                                    pt_t[:, hp * 2 + j, colsl],
                                    msk,
                                )
                    pt_hist[kb] = pt_t
                    if kb >= 2:
                        pt_hist.pop(kb - 2)
                        va_hist.pop(kb - 2)

                # ---------------- stage C+D: attention out + y for qb=g-1 ----------------
                if g >= 1:
                    qb = g - 1
                    sl = (qb % 2) * 128
                    ao = att.tile([128, HL, D], F32, tag="ao")
                    rec = att.tile([128, HL], F32, tag="rec")
                    for hq in range(2):  # head quads
                        o_ps = ps_oy.tile([128, 4, 66], F32, tag="oy_ps", name="o_ps")
                        for hh in range(4):
                            h = hq * 4 + hh
                            if qb >= 1:
                                nc.tensor.matmul(
                                    o_ps[:, hh, :],
                                    pt_hist[qb - 1][:, h, sl : sl + 128],
                                    va_hist[qb - 1][:, h, :],
                                    start=True, stop=False,
                                )
                            nc.tensor.matmul(
                                o_ps[:, hh, :],
                                pt_hist[qb][:, h, sl : sl + 128],
                                va_hist[qb][:, h, :],
                                start=(qb == 0), stop=True,
                            )
                        hsl = slice(hq * 4, hq * 4 + 4)
                        nc.vector.reciprocal(
                            out=rec[:, hsl],
                            in_=o_ps[:, :, 64:65].rearrange("p h o -> p (h o)"),
                        )
                        nc.vector.tensor_tensor(
                            out=ao[:, hsl, :],
                            in0=o_ps[:, :, 0:64],
                            in1=rec[:, hsl].broadcast_to([128, 4, D]),
                            op=ALU.mult,
                        )

                    # transpose ao and project
                    aoT = att.tile([128, HT, 128], F32R, tag="aoT")
                    aov = ao[:].rearrange("p h d -> p (h d)")
                    for ht in range(HT):
                        tp3 = ps_tr.tile([128, 128], F32, tag="tp")
                        nc.tensor.transpose(tp3[:], aov[:, ht * 128 : (ht + 1) * 128], idn_t[:].bitcast(F32))
                        nc.scalar.copy(aoT[:, ht, :], tp3[:])

                    y_sb = io.tile([128, CIN], F32, tag="y_sb")
                    for half in range(2):
                        y_ps = ps_oy.tile([128, 512], F32, tag="oy_ps", name="y_ps")
                        for ct in range(HT):
                            nc.tensor.matmul(
                                y_ps[:],
                                aoT[:, ct, :],
                                wp_t[:, ct, half * 512 : (half + 1) * 512],
                                start=(ct == 0), stop=(ct == HT - 1),
                            )
                        nc.scalar.copy(y_sb[:, half * 512 : (half + 1) * 512], y_ps[:])
                    nc.sync.dma_start(y_d[qb * 128 : (qb + 1) * 128, :], y_sb[:])

    nc.compile()
    return nc


def _host_prep(x, v1, cos, sin, Wq, Wk, Wv, Wproj, lamb, maskmode="mm"):
    B = x.shape[0]
    # chronological band mask: rows=key pos in block kb, cols=256 queries
    tk = np.arange(128)[:, None]
    tq = np.arange(256)[None, :]
    valid = np.where(tq < 128, tq >= tk, (tq - 128) < tk)
    if maskmode == "mm":
        bandb = np.where(valid, 0.0, -1e30).astype(np.float32)
    else:
        bandb = valid.astype(np.float32)
    bandbsw = np.concatenate([bandb[:, 128:], bandb[:, :128]], axis=1)
    idn = np.eye(128, dtype=np.float32)

    in_maps = []
    for core in range(8):
        b, hg = core // 2, core % 2
        rsl = slice(hg * 512, (hg + 1) * 512)
        xT = np.ascontiguousarray(x[b].T)
        wqT = np.ascontiguousarray(Wq[rsl, :].T)
        wkT = np.ascontiguousarray(Wk[rsl, :].T)
        wvT = np.ascontiguousarray((1.0 - lamb) * Wv[rsl, :].T)
        wpT = np.ascontiguousarray(Wproj[:, rsl].T)
        v1s = np.ascontiguousarray(lamb * v1[b].reshape(T, 1024)[:, rsl])
        sinb = sin[b].copy()
        sinb[:, :32] *= -1.0
        in_maps.append({
            "xT": xT, "wqT": wqT, "wkT": wkT, "wvT": wvT, "wpT": wpT,
            "v1s": v1s, "cosb": np.ascontiguousarray(cos[b]),
            "sinb": np.ascontiguousarray(sinb),
            "bandb": bandb, "bandbsw": bandbsw, "idn": idn,
        })
    return in_maps


def kernel(x, v1, cos, sin, Wq, Wk, Wv, Wproj, lamb, max_seq_length=None, **_):
    x = np.asarray(x, dtype=np.float32)
    v1 = np.asarray(v1, dtype=np.float32)
    cos = np.asarray(cos, dtype=np.float32)
    sin = np.asarray(sin, dtype=np.float32)
    Wq = np.asarray(Wq, dtype=np.float32)
    Wk = np.asarray(Wk, dtype=np.float32)
    Wv = np.asarray(Wv, dtype=np.float32)
    Wproj = np.asarray(Wproj, dtype=np.float32)
    lamb = float(np.asarray(lamb))

    if "nc" not in _CACHE:
        _CACHE["nc"] = _build()
    nc = _CACHE["nc"]

    in_maps = _host_prep(x, v1, cos, sin, Wq, Wk, Wv, Wproj, lamb)
    res = run_bass_kernel_spmd(nc, in_maps, core_ids=list(range(8)))

    B = x.shape[0]
    y = np.empty((B, T, CIN), dtype=np.float32)
    for b in range(B):
        y[b] = res.results[2 * b]["y"] + res.results[2 * b + 1]["y"]
    return y, v1


# revision 21
# speedup vs baseline: 1.1842x; 1.0089x over previous
"""Trainium2 Bass kernel for nn_CausalSelfAttention_60224031424653.

Reference computation (B=4, T=4096, C=1024, H=16, D=64, sliding window 128):
  q,k,v = x@Wq.T, x@Wk.T, x@Wv.T ; v = (1-lamb)*v + lamb*v1
  q,k = rms_norm(q), rms_norm(k) (per head, over D)
  q,k = rope(q,k; cos,sin)
  out = sliding-window causal attention (each query sees previous 128 keys)
  y = out@Wproj.T ;  returns (y, v1)

Sharding (8 cores): core c = (batch b = c//2, head-group hg = c%2 of 8 heads).
Each core computes a partial y over its 512 channels' contribution to the
output projection; host sums the two partials per batch.

Per-core kernel: stream T in 128-token blocks g=0..31 with a one-block lag:
  iter g:  A) project block g -> q,k,v (fp32r matmuls, xT stationary);
              rms-norm via DVE Newton rsqrt; rope; PE-transpose q,k
           B) scores ST_{g-1} [keys of block g-1 x 256 queries] + additive
              band mask via identity matmul; exp on ACT -> PT (fp32r)
           C) attention output for query block g-1 from PT_{g-1}, PT_{g-2}
              with a fused ones-column denominator in v_aug; normalize on DVE
           D) output projection of block g-1; DMA partial y out.

fp32r (fp32 rounded to ~13 mantissa bits, exact fp32 PSUM accumulation) is
used for all matmul operands: ~4x faster than plain fp32 on the PE.
"""
import sys

sys.path.insert(0, "/opt/trn_rl_repo")

import numpy as np
import concourse.bass as bass
import concourse.mybir as mybir
import concourse.tile as tile
from concourse import bacc
from concourse.bass_utils import run_bass_kernel_spmd

F32 = mybir.dt.float32
F32R = mybir.dt.float32r
I32 = mybir.dt.int32
ALU = mybir.AluOpType
ACTF = mybir.ActivationFunctionType

T = 4096
CIN = 1024
HL = 8          # local heads per core
D = 64
COUT = HL * D   # 512
NB = T // 128   # 32 blocks
CT = CIN // 128  # 8 cin tiles
HT = COUT // 128  # 4 local cout tiles
EPS64 = 64.0 * 1e-6
MAGIC = 0x5F3759DF

_CACHE = {}


def _build(maskmode="mm"):
    nc = bacc.Bacc("TRN2", target_bir_lowering=False, debug=False, num_devices=8)

    xT_d = nc.dram_tensor("xT", [CIN, T], F32R, kind="ExternalInput").ap()
    wq_d = nc.dram_tensor("wqT", [CIN, COUT], F32R, kind="ExternalInput").ap()
    wk_d = nc.dram_tensor("wkT", [CIN, COUT], F32R, kind="ExternalInput").ap()
    wv_d = nc.dram_tensor("wvT", [CIN, COUT], F32R, kind="ExternalInput").ap()
    wp_d = nc.dram_tensor("wpT", [COUT, CIN], F32R, kind="ExternalInput").ap()
    v1_d = nc.dram_tensor("v1s", [T, COUT], F32, kind="ExternalInput").ap()
    cos_d = nc.dram_tensor("cosb", [T, D], F32, kind="ExternalInput").ap()
    sin_d = nc.dram_tensor("sinb", [T, D], F32, kind="ExternalInput").ap()
    bb_d = nc.dram_tensor("bandb", [128, 256], F32R, kind="ExternalInput").ap()
    bbs_d = nc.dram_tensor("bandbsw", [128, 256], F32R, kind="ExternalInput").ap()
    idn_d = nc.dram_tensor("idn", [128, 128], F32R, kind="ExternalInput").ap()
    y_d = nc.dram_tensor("y", [T, CIN], F32, kind="ExternalOutput").ap()

    xT_v = xT_d.rearrange("(ct p) t -> p ct t", p=128)
    wq_v = wq_d.rearrange("(ct p) n -> p ct n", p=128)
    wk_v = wk_d.rearrange("(ct p) n -> p ct n", p=128)
    wv_v = wv_d.rearrange("(ct p) n -> p ct n", p=128)
    wp_v = wp_d.rearrange("(ct p) n -> p ct n", p=128)

    with tile.TileContext(nc) as tc:
        with (
            tc.tile_pool(name="wpool", bufs=1) as wpool,
            tc.tile_pool(name="ring", bufs=1) as ringp,
            tc.tile_pool(name="io", bufs=3) as io,
            tc.tile_pool(name="stg", bufs=3) as stg,
            tc.tile_pool(name="kv", bufs=3) as kvp,
            tc.tile_pool(name="vau", bufs=4) as vaup,
            tc.tile_pool(name="pt", bufs=3) as ptp,
            tc.tile_pool(name="attn", bufs=3) as att,
            tc.tile_pool(name="ps_qkv", bufs=2, space="PSUM") as ps_qkv,
            tc.tile_pool(name="ps_tr", bufs=2, space="PSUM") as ps_tr,
            tc.tile_pool(name="ps_st", bufs=2, space="PSUM") as ps_st,
            tc.tile_pool(name="ps_oy", bufs=2, space="PSUM") as ps_oy,
        ):
            wq_t = wpool.tile([128, CT, COUT], F32R, name="wq_t")
            wk_t = wpool.tile([128, CT, COUT], F32R, name="wk_t")
            wv_t = wpool.tile([128, CT, COUT], F32R, name="wv_t")
            wp_t = wpool.tile([128, HT, CIN], F32R, name="wp_t")
            bb_t = wpool.tile([128, 256], F32R, name="bb_t")
            bbs_t = wpool.tile([128, 256], F32R, name="bbs_t")
            idn_t = wpool.tile([128, 128], F32R, name="idn_t")
            nc.sync.dma_start(wq_t[:], wq_v)
            nc.sync.dma_start(wk_t[:], wk_v)
            nc.sync.dma_start(wv_t[:], wv_v)
            nc.sync.dma_start(wp_t[:], wp_v)
            nc.sync.dma_start(bb_t[:], bb_d)
            nc.sync.dma_start(bbs_t[:], bbs_d)
            nc.sync.dma_start(idn_t[:], idn_d)

            # qT ring: [d-of-2-heads(128), ht, slot, t]
            qring = ringp.tile([128, HT, 2, 128], F32R, name="qring")

            kT_hist = {}    # g -> kT tile [128, HT, 128]
            va_hist = {}    # g -> v_aug tile [128, HL, 66]
            pt_hist = {}    # g -> PT tile [128, HL, 256]

            for g in range(NB + 1):
                # ---------------- stage A: project block g ----------------
                if g < NB:
                    c0 = g * 128
                    xg = io.tile([128, CT, 128], F32R, tag="xg")
                    nc.sync.dma_start(xg[:], xT_v[:, :, c0 : c0 + 128])
                    v1g = io.tile([128, COUT], F32, tag="v1g")
                    nc.sync.dma_start(v1g[:], v1_d[c0 : c0 + 128, :])
                    cosg = io.tile([128, D], F32, tag="cosg")
                    nc.sync.dma_start(cosg[:], cos_d[c0 : c0 + 128, :])
                    sing = io.tile([128, D], F32, tag="sing")
                    nc.sync.dma_start(sing[:], sin_d[c0 : c0 + 128, :])

                    q_ps = ps_qkv.tile([128, COUT], F32, tag="qkv_ps", name="q_ps")
                    k_ps = ps_qkv.tile([128, COUT], F32, tag="qkv_ps", name="k_ps")
                    v_ps = ps_qkv.tile([128, COUT], F32, tag="qkv_ps", name="v_ps")
                    for ct in range(CT):
                        se = dict(start=(ct == 0), stop=(ct == CT - 1))
                        nc.tensor.matmul(q_ps[:], xg[:, ct, :], wq_t[:, ct, :], **se)
                    for ct in range(CT):
                        se = dict(start=(ct == 0), stop=(ct == CT - 1))
                        nc.tensor.matmul(k_ps[:], xg[:, ct, :], wk_t[:, ct, :], **se)
                    for ct in range(CT):
                        se = dict(start=(ct == 0), stop=(ct == CT - 1))
                        nc.tensor.matmul(v_ps[:], xg[:, ct, :], wv_t[:, ct, :], **se)

                    # v_aug: [t, h, 0:64]=v + v1s (pre-scaled), [t, h, 64:66]=1
                    v_aug = vaup.tile([128, HL, 66], F32R, tag="v_aug")
                    nc.vector.memset(v_aug[:, :, 64:66].bitcast(F32), 1.0)
                    nc.vector.tensor_tensor(
                        out=v_aug[:, :, 0:64],
                        in0=v_ps[:].rearrange("p (h d) -> p h d", h=HL),
                        in1=v1g[:].rearrange("p (h d) -> p h d", h=HL),
                        op=ALU.add,
                    )
                    va_hist[g] = v_aug

                    # copy q,k to SBUF so the PSUM banks free quickly
                    qsb = stg.tile([128, COUT], F32, tag="qsb")
                    ksb = stg.tile([128, COUT], F32, tag="ksb")
                    nc.scalar.copy(qsb[:], q_ps[:])
                    nc.scalar.copy(ksb[:], k_ps[:])
                    # sum of squares for q (cols 0:8) and k (cols 8:16)
                    sq = stg.tile([128, COUT], F32, tag="sq")
                    ssq = stg.tile([128, 2 * HL], F32, tag="ssq")
                    nc.vector.tensor_tensor(out=sq[:], in0=qsb[:], in1=qsb[:], op=ALU.mult)
                    nc.vector.reduce_sum(
                        out=ssq[:, 0:HL],
                        in_=sq[:].rearrange("p (h d) -> p h d", h=HL),
                        axis=mybir.AxisListType.X,
                    )
                    nc.vector.tensor_tensor(out=sq[:], in0=ksb[:], in1=ksb[:], op=ALU.mult)
                    nc.vector.reduce_sum(
                        out=ssq[:, HL : 2 * HL],
                        in_=sq[:].rearrange("p (h d) -> p h d", h=HL),
                        axis=mybir.AxisListType.X,
                    )
                    nc.vector.tensor_scalar_add(ssq[:], in0=ssq[:], scalar1=EPS64)
                    # Newton rsqrt: r = 1/sqrt(ssq); q scale=r, k scale=8r
                    rsy = stg.tile([128, 2 * HL], F32, tag="rsy")
                    rst = stg.tile([128, 2 * HL], F32, tag="rst")
                    nc.vector.tensor_scalar(
                        out=rst[:].bitcast(I32),
                        in0=ssq[:].bitcast(I32),
                        scalar1=1,
                        scalar2=None,
                        op0=ALU.logical_shift_right,
                    )
                    nc.vector.tensor_scalar(
                        out=rsy[:].bitcast(I32),
                        in0=rst[:].bitcast(I32),
                        scalar1=-1,
                        scalar2=None,
                        op0=ALU.bitwise_xor,
                    )
                    nc.vector.tensor_scalar(
                        out=rsy[:].bitcast(I32),
                        in0=rsy[:].bitcast(I32),
                        scalar1=MAGIC + 1,
                        scalar2=None,
                        op0=ALU.add,
                    )
                    for _ in range(3):
                        nc.vector.tensor_tensor(out=rst[:], in0=ssq[:], in1=rsy[:], op=ALU.mult)
                        nc.vector.tensor_tensor(out=rst[:], in0=rst[:], in1=rsy[:], op=ALU.mult)
                        nc.vector.tensor_scalar(
                            out=rst[:], in0=rst[:], scalar1=-0.5, scalar2=1.5,
                            op0=ALU.mult, op1=ALU.add,
                        )
                        nc.vector.tensor_tensor(out=rsy[:], in0=rsy[:], in1=rst[:], op=ALU.mult)

                    # normalize (q also folds the 1/8 attention scale)
                    qn = stg.tile([128, COUT], F32, tag="qn")
                    kn = stg.tile([128, COUT], F32, tag="kn")
                    nc.vector.tensor_tensor(
                        out=qn[:].rearrange("p (h d) -> p h d", h=HL),
                        in0=qsb[:].rearrange("p (h d) -> p h d", h=HL),
                        in1=rsy[:, 0:HL].broadcast_to([128, HL, D]),
                        op=ALU.mult,
                    )
                    nc.vector.scalar_tensor_tensor(
                        out=kn[:].rearrange("p (h d) -> p h d", h=HL),
                        in0=ksb[:].rearrange("p (h d) -> p h d", h=HL),
                        scalar=8.0,
                        in1=rsy[:, HL : 2 * HL].broadcast_to([128, HL, D]),
                        op0=ALU.mult,
                        op1=ALU.mult,
                    )

                    # rope: out = qn*cos + swap_halves(qn)*sin_signed
                    qr = stg.tile([128, COUT], F32, tag="qr")
                    kr = stg.tile([128, COUT], F32, tag="kr")
                    tmp = stg.tile([128, COUT], F32, tag="tmp")
                    for (src, dst) in ((qn, qr), (kn, kr)):
                        sv = src[:].rearrange("p (h a d) -> p h a d", a=2, d=32)
                        swapped = bass.AP(
                            tensor=sv.tensor,
                            offset=sv.offset + 32,
                            ap=[sv.ap[0], sv.ap[1], [-32, 2], [1, 32]],
                        )
                        sing_b = bass.AP(
                            tensor=sing[:].tensor,
                            offset=sing[:].offset,
                            ap=[sing[:].ap[0], [0, HL], [32, 2], [1, 32]],
                        )
                        nc.vector.tensor_tensor(
                            out=tmp[:].rearrange("p (h a d) -> p h a d", a=2, d=32),
                            in0=swapped,
                            in1=sing_b,
                            op=ALU.mult,
                        )
                        cosg_b = bass.AP(
                            tensor=cosg[:].tensor,
                            offset=cosg[:].offset,
                            ap=[cosg[:].ap[0], [0, HL], [1, D]],
                        )
                        nc.vector.tensor_tensor(
                            out=dst[:].rearrange("p (h d) -> p h d", h=HL),
                            in0=src[:].rearrange("p (h d) -> p h d", h=HL),
                            in1=cosg_b,
                            op=ALU.mult,
                        )
                        nc.vector.tensor_tensor(out=dst[:], in0=dst[:], in1=tmp[:], op=ALU.add)

                    # transpose q into ring slot g%2, k into kT tile
                    kT = kvp.tile([128, HT, 128], F32R, tag="kT")
                    for ht in range(HT):
                        tp = ps_tr.tile([128, 128], F32, tag="tp")
                        nc.tensor.transpose(tp[:], qr[:, ht * 128 : (ht + 1) * 128], idn_t[:].bitcast(F32))
                        nc.scalar.copy(qring[:, ht, g % 2, :], tp[:])
                        tp2 = ps_tr.tile([128, 128], F32, tag="tp")
                        nc.tensor.transpose(tp2[:], kr[:, ht * 128 : (ht + 1) * 128], idn_t[:].bitcast(F32))
                        nc.scalar.copy(kT[:, ht, :], tp2[:])
                    kT_hist[g] = kT

                # ---------------- stage B: scores for key block kb=g-1 ----------------
                if g >= 1:
                    kb = g - 1
                    kTb = kT_hist.pop(kb)
                    tail = kb == NB - 1
                    colsl = slice(128, 256) if tail else slice(0, 256)
                    ncols = 128 if tail else 256
                    maskt = bb_t if (kb % 2 == 0) else bbs_t
                    pt_t = ptp.tile([128, HL, 256], F32R, tag="pt_t")
                    for hp in range(HL // 2):
                        st = ps_st.tile([128, 2, 256], F32, tag="st")
                        for j in range(2):
                            h = hp * 2 + j
                            ht, r0 = h // 2, (h % 2) * 64
                            if tail:
                                qrhs = qring[r0 : r0 + 64, ht, 1, :]
                            else:
                                qrhs = qring[r0 : r0 + 64, ht, :, :]
                            nc.tensor.matmul(
                                st[:, j, colsl],
                                kTb[r0 : r0 + 64, ht, :],
                                qrhs,
                                start=True, stop=(maskmode != "mm"),
                            )
                            if maskmode == "mm":
                                nc.tensor.matmul(
                                    st[:, j, colsl],
                                    idn_t[:],
                                    bb_t[:, 0:128] if tail else maskt[:],
                                    start=False, stop=True,
                                )
                        if maskmode == "dve2":
                            scr = ptp.tile([128, 2, 256], F32, tag="scr")
                            nc.scalar.activation(scr[:, :, colsl], st[:, :, colsl], ACTF.Exp)
                            msk = bb_t[:, 0:128] if tail else maskt[:]
                            for j in range(2):
                                nc.vector.tensor_mul(
                                    pt_t[:, hp * 2 + j, colsl],
                                    scr[:, j, colsl],
                                    msk.bitcast(F32),
                                )
                        else:
                            nc.scalar.activation(
                                pt_t[:, hp * 2 : hp * 2 + 2, colsl],
                                st[:, :, colsl],
                                ACTF.Exp,
                            )
                        if maskmode == "dve":
                            msk = bb_t[:, 0:128] if tail else maskt[:]
                            for j in range(2):
                                nc.vector.tensor_mul(
                                    pt_t[:, hp * 2 + j, colsl],
                                    pt_t[:, hp * 2 + j, colsl],
                                    msk,
                                )
                        elif maskmode == "gps":
                            msk = bb_t[:, 0:128] if tail else maskt[:]
                            for j in range(2):
                                nc.gpsimd.tensor_mul(
                                    pt_t[:, hp * 2 + j, colsl],
                                    pt_t[:, hp * 2 + j, colsl],
                                    msk,
                                )
                    pt_hist[kb] = pt_t
                    if kb >= 2:
                        pt_hist.pop(kb - 2)
                        va_hist.pop(kb - 2)

                # ---------------- stage C+D: attention out + y for qb=g-1 ----------------
                if g >= 1:
                    qb = g - 1
                    sl = (qb % 2) * 128
                    ao = att.tile([128, HL, D], F32, tag="ao")
                    rec = att.tile([128, HL], F32, tag="rec")
                    for hq in range(2):  # head quads
                        o_ps = ps_oy.tile([128, 4, 66], F32, tag="oy_ps", name="o_ps")
                        for hh in range(4):
                            h = hq * 4 + hh
                            if qb >= 1:
                                nc.tensor.matmul(
                                    o_ps[:, hh, :],
                                    pt_hist[qb - 1][:, h, sl : sl + 128],
                                    va_hist[qb - 1][:, h, :],
                                    start=True, stop=False,
                                )
                            nc.tensor.matmul(
                                o_ps[:, hh, :],
                                pt_hist[qb][:, h, sl : sl + 128],
                                va_hist[qb][:, h, :],
                                start=(qb == 0), stop=True,
                            )
                        hsl = slice(hq * 4, hq * 4 + 4)
                        nc.vector.reciprocal(
                            out=rec[:, hsl],
                            in_=o_ps[:, :, 64:65].rearrange("p h o -> p (h o)"),
                        )
                        nc.vector.tensor_tensor(
                            out=ao[:, hsl, :],
                            in0=o_ps[:, :, 0:64],
                            in1=rec[:, hsl].broadcast_to([128, 4, D]),
                            op=ALU.mult,
                        )

                    # transpose ao and project
                    aoT = att.tile([128, HT, 128], F32R, tag="aoT")
                    aov = ao[:].rearrange("p h d -> p (h d)")
                    for ht in range(HT):
                        tp3 = ps_tr.tile([128, 128], F32, tag="tp")
                        nc.tensor.transpose(tp3[:], aov[:, ht * 128 : (ht + 1) * 128], idn_t[:].bitcast(F32))
                        nc.scalar.copy(aoT[:, ht, :], tp3[:])

                    y_sb = io.tile([128, CIN], F32, tag="y_sb")
                    for half in range(2):
                        y_ps = ps_oy.tile([128, 512], F32, tag="oy_ps", name="y_ps")
                        for ct in range(HT):
                            nc.tensor.matmul(
                                y_ps[:],
                                aoT[:, ct, :],
                                wp_t[:, ct, half * 512 : (half + 1) * 512],
                                start=(ct == 0), stop=(ct == HT - 1),
                            )
                        nc.scalar.copy(y_sb[:, half * 512 : (half + 1) * 512], y_ps[:])
                    nc.sync.dma_start(y_d[qb * 128 : (qb + 1) * 128, :], y_sb[:])

    nc.compile()
    return nc


def _host_prep(x, v1, cos, sin, Wq, Wk, Wv, Wproj, lamb, maskmode="mm"):
    B = x.shape[0]
    # chronological band mask: rows=key pos in block kb, cols=256 queries
    tk = np.arange(128)[:, None]
    tq = np.arange(256)[None, :]
    valid = np.where(tq < 128, tq >= tk, (tq - 128) < tk)
    if maskmode == "mm":
        bandb = np.where(valid, 0.0, -1e30).astype(np.float32)
    else:
        bandb = valid.astype(np.float32)
    bandbsw = np.concatenate([bandb[:, 128:], bandb[:, :128]], axis=1)
    idn = np.eye(128, dtype=np.float32)

    in_maps = []
    for core in range(8):
        b, hg = core // 2, core % 2
        rsl = slice(hg * 512, (hg + 1) * 512)
        xT = np.ascontiguousarray(x[b].T)
        wqT = np.ascontiguousarray(Wq[rsl, :].T)
        wkT = np.ascontiguousarray(Wk[rsl, :].T)
        wvT = np.ascontiguousarray((1.0 - lamb) * Wv[rsl, :].T)
        wpT = np.ascontiguousarray(Wproj[:, rsl].T)
        v1s = np.ascontiguousarray(lamb * v1[b].reshape(T, 1024)[:, rsl])
        sinb = sin[b].copy()
        sinb[:, :32] *= -1.0
        in_maps.append({
            "xT": xT, "wqT": wqT, "wkT": wkT, "wvT": wvT, "wpT": wpT,
            "v1s": v1s, "cosb": np.ascontiguousarray(cos[b]),
            "sinb": np.ascontiguousarray(sinb),
            "bandb": bandb, "bandbsw": bandbsw, "idn": idn,
        })
    return in_maps


def kernel(x, v1, cos, sin, Wq, Wk, Wv, Wproj, lamb, max_seq_length=None, **_):
    x = np.asarray(x, dtype=np.float32)
    v1 = np.asarray(v1, dtype=np.float32)
    cos = np.asarray(cos, dtype=np.float32)
    sin = np.asarray(sin, dtype=np.float32)
    Wq = np.asarray(Wq, dtype=np.float32)
    Wk = np.asarray(Wk, dtype=np.float32)
    Wv = np.asarray(Wv, dtype=np.float32)
    Wproj = np.asarray(Wproj, dtype=np.float32)
    lamb = float(np.asarray(lamb))

    if "nc" not in _CACHE:
        _CACHE["nc"] = _build()
    nc = _CACHE["nc"]

    in_maps = _host_prep(x, v1, cos, sin, Wq, Wk, Wv, Wproj, lamb)
    res = run_bass_kernel_spmd(nc, in_maps, core_ids=list(range(8)))

    B = x.shape[0]
    y = np.empty((B, T, CIN), dtype=np.float32)
    for b in range(B):
        y[b] = res.results[2 * b]["y"] + res.results[2 * b + 1]["y"]
    return y, v1
